# revision 25
# baseline (speedup 1.0000x reference)
"""Linear attention (silu+1 feature map) MultiHeadAttention for 8x TRN2.

Sharding: data-parallel over batch (B=8 -> 1 batch element per NeuronCore).

Math per core (T=4096, D=1024, H=16, Dh=64), with phi(z) = 1 + s(z),
s(z) = silu(z). Write s_q = silu(scale*q), s_k = silu(scale*k). Then

  kv_h   = phi_k_h^T v_h
         = colsum_v_h                      (rank-1 in e; exact, bf16/fp32)
         + (s_k^T x)_h @ Wv_h^T           (fp8 "G path": replaces v proj)
         + bv_h (x) rowsum(s_k)_h         (rank-1 correction)
  M      = kv^T-blocks @ Wo^T             (block-diag, bf16)
  y^T    = M8^T @ s_q + colsum_M + bo     (fp8; the +1 of phi_q is folded
                                           into colsum_M = ones^T M)

All big GEMMs (q proj, k proj, G = s_k^T x, phase-2) run as fp8-e4m3
DoubleRow matmuls (2x128-row contraction @ 0.5 cyc/row).  Centering the
+1 out of phi keeps fp8 noise confined to the ~12%-magnitude fluctuation
terms; exact colsums are carried in fp32/bf16.  Weights are scaled by 64
before fp8 quantization to clear the e4m3 subnormal floor; the inverse
scale rides the ACT silu drain.
"""

import numpy as np
import ml_dtypes

B, T, D = 8, 4096, 1024
H, DH = 16, 64
SCALE = float(DH ** -0.25)
NCORES = 8
P = 128
DC = D // P            # 8 feature chunks
NG = T // 256          # 16 groups of 256 tokens
WS = 64.0              # fp8 weight prescale

_BF16 = ml_dtypes.bfloat16
_F8 = ml_dtypes.float8_e4m3

_CACHE = {}


def _split_multi_waits(nc):
    """walrus in this container only encodes ONE sync-wait command per
    instruction. Hoist extra waits onto injected same-engine NOPs placed
    immediately before the instruction."""
    import concourse.mybir as mybir

    n_split = 0
    for fn in nc.m.functions:
        for bb in fn.blocks:
            new = []
            changed = False
            for inst in bb.instructions:
                si = inst.sync_info
                waits = list(si.on_wait) if si is not None else []
                if len(waits) > 1:
                    changed = True
                    for j, w in enumerate(waits[:-1]):
                        nop = mybir.InstNoOp(
                            name=f"{inst.name}-sw{j}", ins=[], outs=[]
                        )
                        nop.engine = inst.engine
                        nop.sync_info = mybir.SyncInfo(
                            on_wait=[w], on_update=[]
                        )
                        new.append(nop)
                        n_split += 1
                    inst.sync_info = mybir.SyncInfo(
                        on_wait=[waits[-1]], on_update=list(si.on_update)
                    )
                new.append(inst)
            if changed:
                bb.instructions = new
    return n_split


def _build_program(debug=False):
    import concourse.bass as bass
    import concourse.mybir as mybir
    from concourse.tile import TileContext

    dt = mybir.dt
    AF = mybir.ActivationFunctionType
    DR = mybir.MatmulPerfMode.DoubleRow

    nc = bass.Bass()

    xT8_d = nc.dram_tensor("xT8", [P, DC, T], dt.float8e4, kind="ExternalInput")
    xt8_d = nc.dram_tensor("xt8", [P, NG, 2, D], dt.float8e4, kind="ExternalInput")
    xr8_d = nc.dram_tensor("xr8", [P, NG, 2, D], dt.float8e4, kind="ExternalInput")
    wq8_d = nc.dram_tensor("wq8", [P, DC, D], dt.float8e4, kind="ExternalInput")
    wk8_d = nc.dram_tensor("wk8", [P, DC, D], dt.float8e4, kind="ExternalInput")
    wvT_d = nc.dram_tensor("wvT", [P, DC, D], dt.bfloat16, kind="ExternalInput")
    woT_d = nc.dram_tensor("woT", [P, DC, D], dt.bfloat16, kind="ExternalInput")
    bqs_d = nc.dram_tensor("bqs", [P, DC], dt.float32, kind="ExternalInput")
    bqa_d = nc.dram_tensor("bqa", [P, DC], dt.float32, kind="ExternalInput")
    bvc_d = nc.dram_tensor("bvc", [P, DC], dt.float32, kind="ExternalInput")
    bob_d = nc.dram_tensor("bob", [P, DC], dt.float32, kind="ExternalInput")
    bvr_d = nc.dram_tensor("bvr", [1, D], dt.bfloat16, kind="ExternalInput")
    one8_d = nc.dram_tensor("one8", [P, 2, 16], dt.float8e4, kind="ExternalInput")
    yT_d = nc.dram_tensor("yT", [P, DC, T], dt.bfloat16, kind="ExternalOutput")
    if debug:
        dbg = {
            "sq": nc.dram_tensor("dbg_sq", [P, DC, T], dt.float8e4, kind="ExternalOutput"),
            "sk": nc.dram_tensor("dbg_sk", [P, NG, 2, D], dt.float8e4, kind="ExternalOutput"),
            "gt": nc.dram_tensor("dbg_gt", [P, DC, D], dt.bfloat16, kind="ExternalOutput"),
            "kv": nc.dram_tensor("dbg_kv", [P, DC, P], dt.bfloat16, kind="ExternalOutput"),
            "m8": nc.dram_tensor("dbg_m8", [P, DC, D], dt.float8e4, kind="ExternalOutput"),
            "csx": nc.dram_tensor("dbg_csx", [P, DC], dt.bfloat16, kind="ExternalOutput"),
            "cv": nc.dram_tensor("dbg_cv", [P, DC], dt.float32, kind="ExternalOutput"),
            "rs": nc.dram_tensor("dbg_rs", [1, D], dt.bfloat16, kind="ExternalOutput"),
            "by": nc.dram_tensor("dbg_by", [P, DC], dt.float32, kind="ExternalOutput"),
        }

    with TileContext(nc) as tc:
      with tc.tile_pool(name="persist", bufs=1) as pp:
        bqs_sb = pp.tile([P, DC], dt.float32, tag="bqs")
        bqa_sb = pp.tile([P, DC], dt.float32, tag="bqa")
        bvc_sb = pp.tile([P, DC], dt.float32, tag="bvc")
        bob_sb = pp.tile([P, DC], dt.float32, tag="bob")
        bvr_sb = pp.tile([1, D], dt.bfloat16, tag="bvr")
        one8_sb = pp.tile([P, 2, 16], dt.float8e4, tag="one8")
        zz = pp.tile([1, 640], dt.bfloat16, tag="zz")
        csxb_sb = pp.tile([P, DC], dt.bfloat16, tag="csxb")
        cv_sb = pp.tile([P, DC], dt.float32, tag="cv")
        u_sb = pp.tile([P, DC], dt.float32, tag="u")
        ub_sb = pp.tile([P, DC], dt.bfloat16, tag="ub")
        by_sb = pp.tile([P, DC], dt.float32, tag="by")
        rs_sb = pp.tile([1, D], dt.bfloat16, tag="rs")
        kvch = pp.tile([P, DC, P], dt.bfloat16, tag="kvch")
        m8_sb = pp.tile([P, DC, D], dt.float8e4, tag="m8")
        sq_sb = pp.tile([P, DC, T], dt.float8e4, tag="sq")

        nc.vector.memset(zz[:1, 0:256], 0.0)
        nc.vector.memset(kvch[:], 0.0)

        with tc.tile_pool(name="bigB", bufs=1) as pb:
          sk_sb = pb.tile([P, NG, 2, D], dt.float8e4, tag="sk")
          xt8_sb = pb.tile([P, NG, 2, D], dt.float8e4, tag="xt8")
          gt_sb = pb.tile([P, DC, D], dt.bfloat16, tag="gt")

          with tc.tile_pool(name="ph1w", bufs=1) as pc:
            wq8_sb = pc.tile([P, DC, D], dt.float8e4, tag="wq8")
            wk8_sb = pc.tile([P, DC, D], dt.float8e4, tag="wk8")
            xT8_sb = pc.tile([P, DC, T], dt.float8e4, tag="xT8")

            # ============ phase 1a (k proj) + colsum-x, xr8 scoped ============
            with tc.tile_pool(name="ph1r", bufs=1) as pcr:
                xr8_sb = pcr.tile([P, NG, 2, D], dt.float8e4, tag="xr8")

                # one DMA queue (gpsimd: 25ns/trigger), strict priority order
                nc.gpsimd.dma_start(wk8_sb[:], wk8_d[:])
                nc.gpsimd.dma_start(xT8_sb[:, :, 0:512], xT8_d[:, :, 0:512])
                nc.gpsimd.dma_start(xT8_sb[:, :, 512:1024], xT8_d[:, :, 512:1024])
                nc.gpsimd.dma_start(xT8_sb[:, :, 1024:2048], xT8_d[:, :, 1024:2048])
                nc.gpsimd.dma_start(wq8_sb[:], wq8_d[:])
                nc.gpsimd.dma_start(xT8_sb[:, :, 2048:3072], xT8_d[:, :, 2048:3072])
                nc.gpsimd.dma_start(xT8_sb[:, :, 3072:4096], xT8_d[:, :, 3072:4096])
                nc.gpsimd.dma_start(bqs_sb[:], bqs_d[:])
                nc.gpsimd.dma_start(bqa_sb[:], bqa_d[:])
                nc.gpsimd.dma_start(one8_sb[:], one8_d[:])
                nc.gpsimd.dma_start(bvc_sb[:], bvc_d[:])
                nc.gpsimd.dma_start(bob_sb[:], bob_d[:])
                nc.gpsimd.dma_start(bvr_sb[:], bvr_d[:])
                for qq in range(4):
                    nc.gpsimd.dma_start(
                        xt8_sb[:, qq * 4 : (qq + 1) * 4, :, :],
                        xt8_d[:, qq * 4 : (qq + 1) * 4, :, :],
                    )
                for hh in range(2):
                    nc.gpsimd.dma_start(
                        xr8_sb[:, hh * 8 : (hh + 1) * 8, :, :],
                        xr8_d[:, hh * 8 : (hh + 1) * 8, :, :],
                    )

                with (
                    tc.tile_pool(name="kps", bufs=3, space="PSUM") as kpool,
                    tc.tile_pool(name="csps", bufs=1, space="PSUM") as cspool,
                ):
                    csp = cspool.tile([P, DC], dt.float32, tag="csp")
                    warm = kpool.tile([P, D], dt.float32, tag="kp")
                    for w in range(78):
                        nc.tensor.matmul(
                            warm[:, 0:128], lhsT=zz[:1, :P],
                            rhs=zz[:1, P : P + 128],
                            start=True, stop=True, skip_group_check=True,
                        )

                    def _k_group(g):
                        for i in range(2):
                            kp = kpool.tile([P, D], dt.float32, tag="kp")
                            t0 = g * 256 + i * 128
                            for ds in range(4):
                                for c in range(4):
                                    nc.tensor.matmul(
                                        kp[:, ds * 256 : (ds + 1) * 256],
                                        lhsT=xT8_sb[:, 2 * c : 2 * c + 2, t0 : t0 + 128],
                                        rhs=wk8_sb[:, 2 * c : 2 * c + 2, ds * 256 : (ds + 1) * 256],
                                        start=(c == 0), stop=(c == 3),
                                        perf_mode=DR, skip_group_check=True,
                                    )
                            nc.scalar.activation(
                                sk_sb[:, g, i, :], kp[:], AF.Silu,
                                scale=SCALE / WS,
                            )

                    for g in range(NG):
                        _k_group(g)

                    # colsum_x column: contract tokens against ones; x8 and
                    # the fp8 residual accumulate into one region
                    for cc in range(DC):
                        for g in range(NG):
                            nc.tensor.matmul(
                                csp[:, cc : cc + 1],
                                lhsT=xt8_sb[:, g, :, cc * P : (cc + 1) * P],
                                rhs=one8_sb[:, :, 0:1],
                                start=(g == 0), stop=False,
                                perf_mode=DR, skip_group_check=True,
                            )
                        for g in range(NG):
                            nc.tensor.matmul(
                                csp[:, cc : cc + 1],
                                lhsT=xr8_sb[:, g, :, cc * P : (cc + 1) * P],
                                rhs=one8_sb[:, :, 0:1],
                                start=False, stop=(g == NG - 1),
                                perf_mode=DR, skip_group_check=True,
                            )
                    nc.vector.tensor_copy(out=csxb_sb[:], in_=csp[:])

            # ====== xr8 space free: load wvT/woT during 1b ======
            with tc.tile_pool(name="postw", bufs=1) as pd:
                wvT_sb = pd.tile([P, DC, D], dt.bfloat16, tag="wvT")
                woT_sb = pd.tile([P, DC, D], dt.bfloat16, tag="woT")
                nc.sync.dma_start(wvT_sb[:], wvT_d[:])
                nc.sync.dma_start(woT_sb[:], woT_d[:])

                # ====== phase 1b: q proj interleaved with G half-chunks ======
                with (
                    tc.tile_pool(name="qps", bufs=4, space="PSUM") as qpool,
                    tc.tile_pool(name="gps", bufs=2, space="PSUM") as gpool,
                ):
                    def _q_group(g):
                        for half in range(4):
                            oc0 = half * 2
                            qp = qpool.tile([P, 2, 256], dt.float32, tag="qp")
                            for j in range(2):
                                oc = oc0 + j
                                for c in range(4):
                                    nc.tensor.matmul(
                                        qp[:, j, :],
                                        lhsT=wq8_sb[:, 2 * c : 2 * c + 2, oc * P : (oc + 1) * P],
                                        rhs=xT8_sb[:, 2 * c : 2 * c + 2, g * 256 : (g + 1) * 256],
                                        start=(c == 0), stop=(c == 3),
                                        perf_mode=DR, skip_group_check=True,
                                    )
                            dst = sq_sb[:, oc0 : oc0 + 2, g * 256 : (g + 1) * 256]
                            if half % 2 == 0:
                                # bias fused into ACT silu, per oc
                                for j in range(2):
                                    nc.scalar.activation(
                                        sq_sb[:, oc0 + j, g * 256 : (g + 1) * 256],
                                        qp[:, j, :], AF.Silu,
                                        bias=bqa_sb[:, oc0 + j : oc0 + j + 1],
                                        scale=SCALE / WS,
                                    )
                            else:
                                # bias on DVE (WS-scaled), plain silu on ACT
                                for j in range(2):
                                    nc.vector.tensor_scalar_add(
                                        qp[:, j, :], qp[:, j, :],
                                        bqs_sb[:, oc0 + j : oc0 + j + 1],
                                    )
                                nc.scalar.activation(
                                    dst, qp[:], AF.Silu, scale=SCALE / WS,
                                )

                    g_tiles = {}

                    def _g_half(idx):
                        cc, half = idx // 2, idx % 2
                        if half == 0:
                            gp = gpool.tile([P, D], dt.float32, tag="gp")
                            g_tiles[cc] = gp
                        else:
                            gp = g_tiles[cc]
                        # ds-serial regions: each region's g==0 matmul carries
                        # start=True; a later region's start only re-marks the
                        # bank's zero-flags, which no finished region rewrites
                        for ds in (half * 2, half * 2 + 1):
                            for g in range(NG):
                                nc.tensor.matmul(
                                    gp[:, ds * 256 : (ds + 1) * 256],
                                    lhsT=xt8_sb[:, g, :, cc * P : (cc + 1) * P],
                                    rhs=sk_sb[:, g, :, ds * 256 : (ds + 1) * 256],
                                    start=(g == 0), stop=(g == NG - 1),
                                    perf_mode=DR, skip_group_check=True,
                                )
                        if half == 1:
                            nc.vector.tensor_copy(out=gt_sb[:, cc, :], in_=gp[:])

                    for g in range(NG):
                        _q_group(g)
                        _g_half(g)

                # ---- rowsum(s_k), colsum_v, kv blocks (one psum pool:
                # no pool-exit WAR between them) ----
                with tc.tile_pool(name="rkv", bufs=1, space="PSUM") as rkpool:
                    rsp = rkpool.tile([1, D], dt.float32, tag="rsp")
                    cvp = rkpool.tile([P, DC], dt.float32, tag="cvp")
                    for ds in range(4):
                        for g in range(NG):
                            nc.tensor.matmul(
                                rsp[:, ds * 256 : (ds + 1) * 256],
                                lhsT=one8_sb[:, :, 0:1],
                                rhs=sk_sb[:, g, :, ds * 256 : (ds + 1) * 256],
                                start=(g == 0), stop=(g == NG - 1),
                                perf_mode=DR, skip_group_check=True,
                            )
                    nc.scalar.copy(out=rs_sb[:], in_=rsp[:])
                    # colsum_v = Wv @ colsum_x  (+ T*bv via bvc)
                    for b in range(DC):
                        for cc in range(DC):
                            nc.tensor.matmul(
                                cvp[:, b : b + 1],
                                lhsT=wvT_sb[:, cc, b * P : (b + 1) * P],
                                rhs=csxb_sb[:, cc : cc + 1],
                                start=(cc == 0), stop=(cc == DC - 1),
                                skip_group_check=True,
                            )
                    nc.vector.tensor_add(cv_sb[:], cvp[:], bvc_sb[:])

                # kv blocks: one PSUM bank per block (start=True zeroes a
                # whole 2KB bank region, so sharing banks WAR-serializes on
                # drains)
                with tc.tile_pool(name="kvbk", bufs=1, space="PSUM") as kvpool:
                    for b in range(DC):
                        kvp = kvpool.tile([P, 512], dt.float32,
                                          tag=f"kvp{b}", name=f"kvp{b}")
                        for cc in range(DC):
                            nc.tensor.matmul(
                                kvp[:, 0:P],
                                lhsT=wvT_sb[:, cc, b * P : (b + 1) * P],
                                rhs=gt_sb[:, cc, b * P : (b + 1) * P],
                                start=(cc == 0), stop=False,
                                skip_group_check=True,
                            )
                        nc.tensor.matmul(
                            kvp[:, 0:P],
                            lhsT=bvr_sb[:1, b * P : (b + 1) * P],
                            rhs=rs_sb[:1, b * P : (b + 1) * P],
                            start=False, stop=True, skip_group_check=True,
                        )
                        nc.scalar.activation(
                            kvch[0:64, b, 0:64], kvp[0:64, 0:64],
                            AF.Identity, bias=cv_sb[0:64, b : b + 1],
                        )
                        nc.scalar.activation(
                            kvch[64:128, b, 64:128], kvp[64:128, 64:128],
                            AF.Identity, bias=cv_sb[64:128, b : b + 1],
                        )
                        nc.vector.tensor_reduce(
                            u_sb[0:64, b : b + 1], kvp[0:64, 0:64],
                            axis=mybir.AxisListType.X, op=mybir.AluOpType.add,
                        )
                        nc.vector.tensor_reduce(
                            u_sb[64:128, b : b + 1], kvp[64:128, 64:128],
                            axis=mybir.AxisListType.X, op=mybir.AluOpType.add,
                        )
                    # u includes the cv bias once per within-head column
                    nc.vector.scalar_tensor_tensor(
                        out=ub_sb[:], in0=cv_sb[:], scalar=float(DH),
                        in1=u_sb[:], op0=mybir.AluOpType.mult,
                        op1=mybir.AluOpType.add,
                    )

                # M = kv^T @ Wo^T ; colsum_M
                with (
                    tc.tile_pool(name="mps", bufs=3, space="PSUM") as mpool,
                    tc.tile_pool(name="cmps", bufs=1, space="PSUM") as cmpool,
                ):
                    for b in range(DC):
                        mp = mpool.tile([P, D], dt.float32, tag="mp")
                        for hh in range(2):
                            nc.tensor.matmul(
                                mp[:, hh * 512 : (hh + 1) * 512],
                                lhsT=kvch[:, b, :],
                                rhs=woT_sb[:, b, hh * 512 : (hh + 1) * 512],
                                start=True, stop=True, skip_group_check=True,
                            )
                        if b % 2 == 0:
                            nc.scalar.copy(out=m8_sb[:, b, :], in_=mp[:])
                        else:
                            nc.vector.tensor_copy(out=m8_sb[:, b, :], in_=mp[:])
                    cmp_t = cmpool.tile([P, DC], dt.float32, tag="cmp")
                    for oc in range(DC):
                        for b in range(DC):
                            nc.tensor.matmul(
                                cmp_t[:, oc : oc + 1],
                                lhsT=woT_sb[:, b, oc * P : (oc + 1) * P],
                                rhs=ub_sb[:, b : b + 1],
                                start=(b == 0), stop=(b == DC - 1),
                                skip_group_check=True,
                            )
                    nc.vector.tensor_add(by_sb[:], cmp_t[:], bob_sb[:])

                if debug:
                    nc.sync.dma_start(dbg["sq"][:], sq_sb[:])
                    nc.sync.dma_start(dbg["sk"][:], sk_sb[:])
                    nc.sync.dma_start(dbg["gt"][:], gt_sb[:])
                    nc.sync.dma_start(dbg["kv"][:], kvch[:])
                    nc.sync.dma_start(dbg["m8"][:], m8_sb[:])
                    nc.sync.dma_start(dbg["csx"][:], csxb_sb[:])
                    nc.sync.dma_start(dbg["cv"][:], cv_sb[:])
                    nc.sync.dma_start(dbg["rs"][:], rs_sb[:])
                    nc.sync.dma_start(dbg["by"][:], by_sb[:])

        # ================= phase 2: y^T = M8^T s_q + bias =================
        with (
            tc.tile_pool(name="yout", bufs=6) as ypool,
            tc.tile_pool(name="yps", bufs=8, space="PSUM") as ypsp,
        ):
            n = 0
            for oc in range(DC):
                for tp in range(8):
                    if tp % 2 == 0:
                        ys = ypool.tile(
                            [P, 2, 512], dt.bfloat16, tag="ys", name="ys"
                        )
                    yp = ypsp.tile([P, 512], dt.float32, tag="yp")
                    for hh in range(2):
                        ts = tp * 2 + hh
                        for f in range(4):
                            nc.tensor.matmul(
                                yp[:, hh * 256 : (hh + 1) * 256],
                                lhsT=m8_sb[:, 2 * f : 2 * f + 2, oc * P : (oc + 1) * P],
                                rhs=sq_sb[:, 2 * f : 2 * f + 2, ts * 256 : (ts + 1) * 256],
                                start=(f == 0), stop=(f == 3),
                                perf_mode=DR, skip_group_check=True,
                            )
                    if n % 2 == 0:
                        nc.scalar.activation(
                            ys[:, tp % 2, :], yp[:], AF.Identity,
                            bias=by_sb[:, oc : oc + 1], scale=1.0,
                        )
                    else:
                        nc.vector.tensor_scalar_add(
                            ys[:, tp % 2, :], yp[:], by_sb[:, oc : oc + 1]
                        )
                    if oc == DC - 1:
                        # tail: small per-tile transfers, all on sync HWDGE
                        nc.sync.dma_start(
                            yT_d[:, oc, tp * 512 : (tp + 1) * 512],
                            ys[:, tp % 2, :],
                        )
                    elif tp % 2 == 1:
                        q = nc.sync if (n // 2) % 2 == 0 else nc.gpsimd
                        q.dma_start(
                            yT_d[:, oc, (tp - 1) * 512 : (tp + 1) * 512],
                            ys[:],
                        )
                    n += 1

    _split_multi_waits(nc)
    return nc


def _get_program(debug=False):
    key = ("nc", debug)
    if key not in _CACHE:
        _CACHE[key] = _build_program(debug)
    return _CACHE[key]


def _prep_shared(Wq, bq, Wk, Wv, bv, Wo, bo):
    def wchunk(w, dtype, scale=1.0):
        # [D, D] row-major (contract, out) -> [P, DC, D] with c = cc*128+p
        return np.ascontiguousarray(
            (w * scale).T.reshape(DC, P, D).transpose(1, 0, 2)
        ).astype(dtype)

    shared = {
        "wq8": wchunk(Wq, _F8, WS),
        "wk8": wchunk(Wk, _F8, WS),
        "wvT": wchunk(Wv, _BF16),
        "woT": wchunk(Wo, _BF16),
        # DVE/ACT pre-add this to the WS-scaled q PSUM; ACT then multiplies
        # by SCALE/WS, so the bias carries WS (not SCALE).
        "bqs": np.ascontiguousarray((WS * bq).astype(np.float32).reshape(DC, P).T),
        "bqa": np.ascontiguousarray((SCALE * bq).astype(np.float32).reshape(DC, P).T),
        "bvc": np.ascontiguousarray((T * bv).astype(np.float32).reshape(DC, P).T),
        "bob": np.ascontiguousarray(bo.astype(np.float32).reshape(DC, P).T),
        "bvr": bv.astype(_BF16)[None, :],
        "one8": np.ones((P, 2, 16), _F8),
    }
    return shared


def _prep_x(xb):
    xT = np.ascontiguousarray(xb.T)  # [D, T]
    x8 = xb.astype(_F8)
    xr8 = (xb - x8.astype(np.float32)).astype(_F8)

    def tok(a):
        return np.ascontiguousarray(
            a.reshape(NG, 2, P, D).transpose(2, 0, 1, 3)
        )

    return {
        "xT8": np.ascontiguousarray(
            xT.reshape(DC, P, T).transpose(1, 0, 2)
        ).astype(_F8),
        "xt8": tok(x8),
        "xr8": tok(xr8),
    }


def _run(in_maps, trace=False, debug=False, **kw):
    from concourse.bass_utils import run_bass_kernel_spmd

    nc = _get_program(debug)
    return run_bass_kernel_spmd(nc, in_maps, list(range(len(in_maps))), trace=trace, **kw)


def kernel(x, Wq, bq, Wk, Wv, bv, Wo, bo):
    x = np.asarray(x, dtype=np.float32)
    assert x.shape == (B, T, D), x.shape
    shared = _prep_shared(
        np.asarray(Wq, np.float32), np.asarray(bq, np.float32),
        np.asarray(Wk, np.float32), np.asarray(Wv, np.float32),
        np.asarray(bv, np.float32), np.asarray(Wo, np.float32),
        np.asarray(bo, np.float32),
    )
    in_maps = []
    for b in range(B):
        m = dict(shared)
        m.update(_prep_x(x[b]))
        in_maps.append(m)

    res = _run(in_maps)
    out = np.empty((B, T, D), np.float32)
    for b in range(B):
        yT = np.asarray(res.results[b]["yT"]).astype(np.float32)  # [P, DC, T]
        out[b] = yT.transpose(1, 0, 2).reshape(D, T).T
    return out


# revision 26
# speedup vs baseline: 1.0090x; 1.0090x over previous
"""Linear attention (silu+1 feature map) MultiHeadAttention for 8x TRN2.

Sharding: data-parallel over batch (B=8 -> 1 batch element per NeuronCore).

Math per core (T=4096, D=1024, H=16, Dh=64), with phi(z) = 1 + s(z),
s(z) = silu(z). Write s_q = silu(scale*q), s_k = silu(scale*k). Then

  kv_h   = phi_k_h^T v_h
         = colsum_v_h                      (rank-1 in e; exact, bf16/fp32)
         + (s_k^T x)_h @ Wv_h^T           (fp8 "G path": replaces v proj)
         + bv_h (x) rowsum(s_k)_h         (rank-1 correction)
  M      = kv^T-blocks @ Wo^T             (block-diag, bf16)
  y^T    = M8^T @ s_q + colsum_M + bo     (fp8; the +1 of phi_q is folded
                                           into colsum_M = ones^T M)

All big GEMMs (q proj, k proj, G = s_k^T x, phase-2) run as fp8-e4m3
DoubleRow matmuls (2x128-row contraction @ 0.5 cyc/row).  Centering the
+1 out of phi keeps fp8 noise confined to the ~12%-magnitude fluctuation
terms; exact colsums are carried in fp32/bf16.  Weights are scaled by 64
before fp8 quantization to clear the e4m3 subnormal floor; the inverse
scale rides the ACT silu drain.
"""

import numpy as np
import ml_dtypes

B, T, D = 8, 4096, 1024
H, DH = 16, 64
SCALE = float(DH ** -0.25)
NCORES = 8
P = 128
DC = D // P            # 8 feature chunks
NG = T // 256          # 16 groups of 256 tokens
WS = 64.0              # fp8 weight prescale

_BF16 = ml_dtypes.bfloat16
_F8 = ml_dtypes.float8_e4m3

_CACHE = {}


def _split_multi_waits(nc):
    """walrus in this container only encodes ONE sync-wait command per
    instruction. Hoist extra waits onto injected same-engine NOPs placed
    immediately before the instruction."""
    import concourse.mybir as mybir

    n_split = 0
    for fn in nc.m.functions:
        for bb in fn.blocks:
            new = []
            changed = False
            for inst in bb.instructions:
                si = inst.sync_info
                waits = list(si.on_wait) if si is not None else []
                if len(waits) > 1:
                    changed = True
                    for j, w in enumerate(waits[:-1]):
                        nop = mybir.InstNoOp(
                            name=f"{inst.name}-sw{j}", ins=[], outs=[]
                        )
                        nop.engine = inst.engine
                        nop.sync_info = mybir.SyncInfo(
                            on_wait=[w], on_update=[]
                        )
                        new.append(nop)
                        n_split += 1
                    inst.sync_info = mybir.SyncInfo(
                        on_wait=[waits[-1]], on_update=list(si.on_update)
                    )
                new.append(inst)
            if changed:
                bb.instructions = new
    return n_split


def _build_program(debug=False):
    import concourse.bass as bass
    import concourse.mybir as mybir
    from concourse.tile import TileContext

    dt = mybir.dt
    AF = mybir.ActivationFunctionType
    DR = mybir.MatmulPerfMode.DoubleRow

    nc = bass.Bass()

    xT8_d = nc.dram_tensor("xT8", [P, DC, T], dt.float8e4, kind="ExternalInput")
    xt8_d = nc.dram_tensor("xt8", [P, NG, 2, D], dt.float8e4, kind="ExternalInput")
    xr8_d = nc.dram_tensor("xr8", [P, NG, 2, D], dt.float8e4, kind="ExternalInput")
    wq8_d = nc.dram_tensor("wq8", [P, DC, D], dt.float8e4, kind="ExternalInput")
    wk8_d = nc.dram_tensor("wk8", [P, DC, D], dt.float8e4, kind="ExternalInput")
    wvT_d = nc.dram_tensor("wvT", [P, DC, D], dt.bfloat16, kind="ExternalInput")
    woT_d = nc.dram_tensor("woT", [P, DC, D], dt.bfloat16, kind="ExternalInput")
    bqs_d = nc.dram_tensor("bqs", [P, DC], dt.float32, kind="ExternalInput")
    bqa_d = nc.dram_tensor("bqa", [P, DC], dt.float32, kind="ExternalInput")
    bvc_d = nc.dram_tensor("bvc", [P, DC], dt.float32, kind="ExternalInput")
    bob_d = nc.dram_tensor("bob", [P, DC], dt.float32, kind="ExternalInput")
    bvr_d = nc.dram_tensor("bvr", [1, D], dt.bfloat16, kind="ExternalInput")
    one8_d = nc.dram_tensor("one8", [P, 2, 16], dt.float8e4, kind="ExternalInput")
    yT_d = nc.dram_tensor("yT", [P, DC, T], dt.bfloat16, kind="ExternalOutput")
    if debug:
        dbg = {
            "sq": nc.dram_tensor("dbg_sq", [P, DC, T], dt.float8e4, kind="ExternalOutput"),
            "sk": nc.dram_tensor("dbg_sk", [P, NG, 2, D], dt.float8e4, kind="ExternalOutput"),
            "gt": nc.dram_tensor("dbg_gt", [P, DC, D], dt.bfloat16, kind="ExternalOutput"),
            "kv": nc.dram_tensor("dbg_kv", [P, DC, P], dt.bfloat16, kind="ExternalOutput"),
            "m8": nc.dram_tensor("dbg_m8", [P, DC, D], dt.float8e4, kind="ExternalOutput"),
            "csx": nc.dram_tensor("dbg_csx", [P, DC], dt.bfloat16, kind="ExternalOutput"),
            "cv": nc.dram_tensor("dbg_cv", [P, DC], dt.float32, kind="ExternalOutput"),
            "rs": nc.dram_tensor("dbg_rs", [1, D], dt.bfloat16, kind="ExternalOutput"),
            "by": nc.dram_tensor("dbg_by", [P, DC], dt.float32, kind="ExternalOutput"),
        }

    with TileContext(nc) as tc:
      with tc.tile_pool(name="persist", bufs=1) as pp:
        bqs_sb = pp.tile([P, DC], dt.float32, tag="bqs")
        bqa_sb = pp.tile([P, DC], dt.float32, tag="bqa")
        bvc_sb = pp.tile([P, DC], dt.float32, tag="bvc")
        bob_sb = pp.tile([P, DC], dt.float32, tag="bob")
        bvr_sb = pp.tile([1, D], dt.bfloat16, tag="bvr")
        one8_sb = pp.tile([P, 2, 16], dt.float8e4, tag="one8")
        zz = pp.tile([1, 640], dt.bfloat16, tag="zz")
        csxb_sb = pp.tile([P, DC], dt.bfloat16, tag="csxb")
        cv_sb = pp.tile([P, DC], dt.float32, tag="cv")
        u_sb = pp.tile([P, DC], dt.float32, tag="u")
        ub_sb = pp.tile([P, DC], dt.bfloat16, tag="ub")
        by_sb = pp.tile([P, DC], dt.float32, tag="by")
        rs_sb = pp.tile([1, D], dt.bfloat16, tag="rs")
        kvch = pp.tile([P, DC, P], dt.bfloat16, tag="kvch")
        m8_sb = pp.tile([P, DC, D], dt.float8e4, tag="m8")
        sq_sb = pp.tile([P, DC, T], dt.float8e4, tag="sq")

        nc.vector.memset(zz[:1, 0:256], 0.0)
        nc.vector.memset(kvch[:], 0.0)

        with tc.tile_pool(name="bigB", bufs=1) as pb:
          sk_sb = pb.tile([P, NG, 2, D], dt.float8e4, tag="sk")
          xt8_sb = pb.tile([P, NG, 2, D], dt.float8e4, tag="xt8")
          gt_sb = pb.tile([P, DC, D], dt.bfloat16, tag="gt")

          with tc.tile_pool(name="ph1w", bufs=1) as pc:
            wq8_sb = pc.tile([P, DC, D], dt.float8e4, tag="wq8")
            wk8_sb = pc.tile([P, DC, D], dt.float8e4, tag="wk8")
            xT8_sb = pc.tile([P, DC, T], dt.float8e4, tag="xT8")

            # ============ phase 1a (k proj) + colsum-x, xr8 scoped ============
            with tc.tile_pool(name="ph1r", bufs=1) as pcr:
                xr8_sb = pcr.tile([P, NG, 2, D], dt.float8e4, tag="xr8")

                # one DMA queue (gpsimd: 25ns/trigger), strict priority order
                nc.gpsimd.dma_start(wk8_sb[:], wk8_d[:])
                nc.gpsimd.dma_start(xT8_sb[:, :, 0:512], xT8_d[:, :, 0:512])
                nc.gpsimd.dma_start(xT8_sb[:, :, 512:1024], xT8_d[:, :, 512:1024])
                nc.gpsimd.dma_start(xT8_sb[:, :, 1024:2048], xT8_d[:, :, 1024:2048])
                nc.gpsimd.dma_start(wq8_sb[:], wq8_d[:])
                nc.gpsimd.dma_start(xT8_sb[:, :, 2048:3072], xT8_d[:, :, 2048:3072])
                nc.gpsimd.dma_start(xT8_sb[:, :, 3072:4096], xT8_d[:, :, 3072:4096])
                nc.gpsimd.dma_start(bqs_sb[:], bqs_d[:])
                nc.gpsimd.dma_start(bqa_sb[:], bqa_d[:])
                nc.gpsimd.dma_start(one8_sb[:], one8_d[:])
                nc.gpsimd.dma_start(bvc_sb[:], bvc_d[:])
                nc.gpsimd.dma_start(bob_sb[:], bob_d[:])
                nc.gpsimd.dma_start(bvr_sb[:], bvr_d[:])
                for qq in range(4):
                    nc.gpsimd.dma_start(
                        xt8_sb[:, qq * 4 : (qq + 1) * 4, :, :],
                        xt8_d[:, qq * 4 : (qq + 1) * 4, :, :],
                    )
                for hh in range(2):
                    nc.gpsimd.dma_start(
                        xr8_sb[:, hh * 8 : (hh + 1) * 8, :, :],
                        xr8_d[:, hh * 8 : (hh + 1) * 8, :, :],
                    )

                with (
                    tc.tile_pool(name="kps", bufs=3, space="PSUM") as kpool,
                    tc.tile_pool(name="csps", bufs=1, space="PSUM") as cspool,
                ):
                    csp = cspool.tile([P, DC], dt.float32, tag="csp")
                    warm = kpool.tile([P, D], dt.float32, tag="kp")
                    for w in range(78):
                        nc.tensor.matmul(
                            warm[:, 0:128], lhsT=zz[:1, :P],
                            rhs=zz[:1, P : P + 128],
                            start=True, stop=True, skip_group_check=True,
                        )

                    def _k_group(g):
                        for i in range(2):
                            kp = kpool.tile([P, D], dt.float32, tag="kp")
                            t0 = g * 256 + i * 128
                            for ds in range(4):
                                for c in range(4):
                                    nc.tensor.matmul(
                                        kp[:, ds * 256 : (ds + 1) * 256],
                                        lhsT=xT8_sb[:, 2 * c : 2 * c + 2, t0 : t0 + 128],
                                        rhs=wk8_sb[:, 2 * c : 2 * c + 2, ds * 256 : (ds + 1) * 256],
                                        start=(c == 0), stop=(c == 3),
                                        perf_mode=DR, skip_group_check=True,
                                    )
                            nc.scalar.activation(
                                sk_sb[:, g, i, :], kp[:], AF.Silu,
                                scale=SCALE / WS,
                            )

                    for g in range(NG):
                        _k_group(g)

                    # colsum_x column: contract tokens against ones; x8 and
                    # the fp8 residual accumulate into one region
                    for cc in range(DC):
                        for g in range(NG):
                            nc.tensor.matmul(
                                csp[:, cc : cc + 1],
                                lhsT=xt8_sb[:, g, :, cc * P : (cc + 1) * P],
                                rhs=one8_sb[:, :, 0:1],
                                start=(g == 0), stop=False,
                                perf_mode=DR, skip_group_check=True,
                            )
                        for g in range(NG):
                            nc.tensor.matmul(
                                csp[:, cc : cc + 1],
                                lhsT=xr8_sb[:, g, :, cc * P : (cc + 1) * P],
                                rhs=one8_sb[:, :, 0:1],
                                start=False, stop=(g == NG - 1),
                                perf_mode=DR, skip_group_check=True,
                            )
                    nc.vector.tensor_copy(out=csxb_sb[:], in_=csp[:])

            # ====== xr8 space free: load wvT/woT during 1b ======
            with tc.tile_pool(name="postw", bufs=1) as pd:
                wvT_sb = pd.tile([P, DC, D], dt.bfloat16, tag="wvT")
                woT_sb = pd.tile([P, DC, D], dt.bfloat16, tag="woT")
                nc.sync.dma_start(wvT_sb[:], wvT_d[:])
                nc.sync.dma_start(woT_sb[:], woT_d[:])

                # ====== phase 1b: q proj interleaved with G half-chunks ======
                with (
                    tc.tile_pool(name="qps", bufs=4, space="PSUM") as qpool,
                    tc.tile_pool(name="gps", bufs=2, space="PSUM") as gpool,
                ):
                    def _q_group(g):
                        for half in range(4):
                            oc0 = half * 2
                            qp = qpool.tile([P, 2, 256], dt.float32, tag="qp")
                            for j in range(2):
                                oc = oc0 + j
                                for c in range(4):
                                    nc.tensor.matmul(
                                        qp[:, j, :],
                                        lhsT=wq8_sb[:, 2 * c : 2 * c + 2, oc * P : (oc + 1) * P],
                                        rhs=xT8_sb[:, 2 * c : 2 * c + 2, g * 256 : (g + 1) * 256],
                                        start=(c == 0), stop=(c == 3),
                                        perf_mode=DR, skip_group_check=True,
                                    )
                            dst = sq_sb[:, oc0 : oc0 + 2, g * 256 : (g + 1) * 256]
                            if half % 2 == 0:
                                # bias fused into ACT silu, per oc
                                for j in range(2):
                                    nc.scalar.activation(
                                        sq_sb[:, oc0 + j, g * 256 : (g + 1) * 256],
                                        qp[:, j, :], AF.Silu,
                                        bias=bqa_sb[:, oc0 + j : oc0 + j + 1],
                                        scale=SCALE / WS,
                                    )
                            else:
                                # bias on DVE (WS-scaled), plain silu on ACT
                                for j in range(2):
                                    nc.vector.tensor_scalar_add(
                                        qp[:, j, :], qp[:, j, :],
                                        bqs_sb[:, oc0 + j : oc0 + j + 1],
                                    )
                                nc.scalar.activation(
                                    dst, qp[:], AF.Silu, scale=SCALE / WS,
                                )

                    g_tiles = {}

                    def _g_half(idx):
                        cc, half = idx // 2, idx % 2
                        if half == 0:
                            gp = gpool.tile([P, D], dt.float32, tag="gp")
                            g_tiles[cc] = gp
                        else:
                            gp = g_tiles[cc]
                        # ds-serial regions: each region's g==0 matmul carries
                        # start=True; a later region's start only re-marks the
                        # bank's zero-flags, which no finished region rewrites
                        for ds in (half * 2, half * 2 + 1):
                            for g in range(NG):
                                nc.tensor.matmul(
                                    gp[:, ds * 256 : (ds + 1) * 256],
                                    lhsT=xt8_sb[:, g, :, cc * P : (cc + 1) * P],
                                    rhs=sk_sb[:, g, :, ds * 256 : (ds + 1) * 256],
                                    start=(g == 0), stop=(g == NG - 1),
                                    perf_mode=DR, skip_group_check=True,
                                )
                        if half == 1:
                            nc.vector.tensor_copy(out=gt_sb[:, cc, :], in_=gp[:])

                    for g in range(NG):
                        _q_group(g)
                        _g_half(g)

                # ---- rowsum(s_k), colsum_v, kv blocks (one psum pool:
                # no pool-exit WAR between them) ----
                with tc.tile_pool(name="rkv", bufs=1, space="PSUM") as rkpool:
                    rsp = rkpool.tile([1, D], dt.float32, tag="rsp")
                    cvp = rkpool.tile([P, DC], dt.float32, tag="cvp")
                    for ds in range(4):
                        for g in range(NG):
                            nc.tensor.matmul(
                                rsp[:, ds * 256 : (ds + 1) * 256],
                                lhsT=one8_sb[:, :, 0:1],
                                rhs=sk_sb[:, g, :, ds * 256 : (ds + 1) * 256],
                                start=(g == 0), stop=(g == NG - 1),
                                perf_mode=DR, skip_group_check=True,
                            )
                    nc.scalar.copy(out=rs_sb[:], in_=rsp[:])
                    # colsum_v = Wv @ colsum_x  (+ T*bv via bvc)
                    for b in range(DC):
                        for cc in range(DC):
                            nc.tensor.matmul(
                                cvp[:, b : b + 1],
                                lhsT=wvT_sb[:, cc, b * P : (b + 1) * P],
                                rhs=csxb_sb[:, cc : cc + 1],
                                start=(cc == 0), stop=(cc == DC - 1),
                                skip_group_check=True,
                            )
                    nc.vector.tensor_add(cv_sb[:], cvp[:], bvc_sb[:])

                # kv blocks: one PSUM bank per block (start=True zeroes a
                # whole 2KB bank region, so sharing banks WAR-serializes on
                # drains)
                with tc.tile_pool(name="kvbk", bufs=1, space="PSUM") as kvpool:
                    for b in range(DC):
                        kvp = kvpool.tile([P, 512], dt.float32,
                                          tag=f"kvp{b}", name=f"kvp{b}")
                        for cc in range(DC):
                            nc.tensor.matmul(
                                kvp[:, 0:P],
                                lhsT=wvT_sb[:, cc, b * P : (b + 1) * P],
                                rhs=gt_sb[:, cc, b * P : (b + 1) * P],
                                start=(cc == 0), stop=False,
                                skip_group_check=True,
                            )
                        nc.tensor.matmul(
                            kvp[:, 0:P],
                            lhsT=bvr_sb[:1, b * P : (b + 1) * P],
                            rhs=rs_sb[:1, b * P : (b + 1) * P],
                            start=False, stop=True, skip_group_check=True,
                        )
                        nc.scalar.activation(
                            kvch[0:64, b, 0:64], kvp[0:64, 0:64],
                            AF.Identity, bias=cv_sb[0:64, b : b + 1],
                        )
                        nc.scalar.activation(
                            kvch[64:128, b, 64:128], kvp[64:128, 64:128],
                            AF.Identity, bias=cv_sb[64:128, b : b + 1],
                        )
                        nc.vector.tensor_reduce(
                            u_sb[0:64, b : b + 1], kvp[0:64, 0:64],
                            axis=mybir.AxisListType.X, op=mybir.AluOpType.add,
                        )
                        nc.vector.tensor_reduce(
                            u_sb[64:128, b : b + 1], kvp[64:128, 64:128],
                            axis=mybir.AxisListType.X, op=mybir.AluOpType.add,
                        )
                    # u includes the cv bias once per within-head column
                    nc.vector.scalar_tensor_tensor(
                        out=ub_sb[:], in0=cv_sb[:], scalar=float(DH),
                        in1=u_sb[:], op0=mybir.AluOpType.mult,
                        op1=mybir.AluOpType.add,
                    )

                # M = kv^T @ Wo^T ; colsum_M
                with (
                    tc.tile_pool(name="mps", bufs=3, space="PSUM") as mpool,
                    tc.tile_pool(name="cmps", bufs=1, space="PSUM") as cmpool,
                ):
                    for b in range(DC):
                        mp = mpool.tile([P, D], dt.float32, tag="mp")
                        for hh in range(2):
                            nc.tensor.matmul(
                                mp[:, hh * 512 : (hh + 1) * 512],
                                lhsT=kvch[:, b, :],
                                rhs=woT_sb[:, b, hh * 512 : (hh + 1) * 512],
                                start=True, stop=True, skip_group_check=True,
                            )
                        if b % 2 == 0:
                            nc.scalar.copy(out=m8_sb[:, b, :], in_=mp[:])
                        else:
                            nc.vector.tensor_copy(out=m8_sb[:, b, :], in_=mp[:])
                    cmp_t = cmpool.tile([P, DC], dt.float32, tag="cmp")
                    for oc in range(DC):
                        for b in range(DC):
                            nc.tensor.matmul(
                                cmp_t[:, oc : oc + 1],
                                lhsT=woT_sb[:, b, oc * P : (oc + 1) * P],
                                rhs=ub_sb[:, b : b + 1],
                                start=(b == 0), stop=(b == DC - 1),
                                skip_group_check=True,
                            )
                    nc.vector.tensor_add(by_sb[:], cmp_t[:], bob_sb[:])

                if debug:
                    nc.sync.dma_start(dbg["sq"][:], sq_sb[:])
                    nc.sync.dma_start(dbg["sk"][:], sk_sb[:])
                    nc.sync.dma_start(dbg["gt"][:], gt_sb[:])
                    nc.sync.dma_start(dbg["kv"][:], kvch[:])
                    nc.sync.dma_start(dbg["m8"][:], m8_sb[:])
                    nc.sync.dma_start(dbg["csx"][:], csxb_sb[:])
                    nc.sync.dma_start(dbg["cv"][:], cv_sb[:])
                    nc.sync.dma_start(dbg["rs"][:], rs_sb[:])
                    nc.sync.dma_start(dbg["by"][:], by_sb[:])

        # ================= phase 2: y^T = M8^T s_q + bias =================
        with (
            tc.tile_pool(name="yout", bufs=6) as ypool,
            tc.tile_pool(name="yps", bufs=8, space="PSUM") as ypsp,
        ):
            n = 0
            for oc in range(DC):
                for tp in range(8):
                    if tp % 2 == 0:
                        ys = ypool.tile(
                            [P, 2, 512], dt.bfloat16, tag="ys", name="ys"
                        )
                    yp = ypsp.tile([P, 512], dt.float32, tag="yp")
                    for hh in range(2):
                        ts = tp * 2 + hh
                        for f in range(4):
                            nc.tensor.matmul(
                                yp[:, hh * 256 : (hh + 1) * 256],
                                lhsT=m8_sb[:, 2 * f : 2 * f + 2, oc * P : (oc + 1) * P],
                                rhs=sq_sb[:, 2 * f : 2 * f + 2, ts * 256 : (ts + 1) * 256],
                                start=(f == 0), stop=(f == 3),
                                perf_mode=DR, skip_group_check=True,
                            )
                    if n % 2 == 0:
                        nc.scalar.activation(
                            ys[:, tp % 2, :], yp[:], AF.Identity,
                            bias=by_sb[:, oc : oc + 1], scale=1.0,
                        )
                    else:
                        nc.vector.tensor_scalar_add(
                            ys[:, tp % 2, :], yp[:], by_sb[:, oc : oc + 1]
                        )
                    if tp % 2 == 1:
                        # keep the last transfers on sync (HWDGE beats the
                        # Pool SWDGE's 1us desc-gen at the kernel tail)
                        q = nc.sync if ((n // 2) % 2 == 0 or n >= 56) \
                            else nc.gpsimd
                        q.dma_start(
                            yT_d[:, oc, (tp - 1) * 512 : (tp + 1) * 512],
                            ys[:],
                        )
                    n += 1

    _split_multi_waits(nc)
    return nc


def _get_program(debug=False):
    key = ("nc", debug)
    if key not in _CACHE:
        _CACHE[key] = _build_program(debug)
    return _CACHE[key]


def _prep_shared(Wq, bq, Wk, Wv, bv, Wo, bo):
    def wchunk(w, dtype, scale=1.0):
        # [D, D] row-major (contract, out) -> [P, DC, D] with c = cc*128+p
        return np.ascontiguousarray(
            (w * scale).T.reshape(DC, P, D).transpose(1, 0, 2)
        ).astype(dtype)

    shared = {
        "wq8": wchunk(Wq, _F8, WS),
        "wk8": wchunk(Wk, _F8, WS),
        "wvT": wchunk(Wv, _BF16),
        "woT": wchunk(Wo, _BF16),
        # DVE/ACT pre-add this to the WS-scaled q PSUM; ACT then multiplies
        # by SCALE/WS, so the bias carries WS (not SCALE).
        "bqs": np.ascontiguousarray((WS * bq).astype(np.float32).reshape(DC, P).T),
        "bqa": np.ascontiguousarray((SCALE * bq).astype(np.float32).reshape(DC, P).T),
        "bvc": np.ascontiguousarray((T * bv).astype(np.float32).reshape(DC, P).T),
        "bob": np.ascontiguousarray(bo.astype(np.float32).reshape(DC, P).T),
        "bvr": bv.astype(_BF16)[None, :],
        "one8": np.ones((P, 2, 16), _F8),
    }
    return shared


def _prep_x(xb):
    xT = np.ascontiguousarray(xb.T)  # [D, T]
    x8 = xb.astype(_F8)
    xr8 = (xb - x8.astype(np.float32)).astype(_F8)

    def tok(a):
        return np.ascontiguousarray(
            a.reshape(NG, 2, P, D).transpose(2, 0, 1, 3)
        )

    return {
        "xT8": np.ascontiguousarray(
            xT.reshape(DC, P, T).transpose(1, 0, 2)
        ).astype(_F8),
        "xt8": tok(x8),
        "xr8": tok(xr8),
    }


def _run(in_maps, trace=False, debug=False, **kw):
    from concourse.bass_utils import run_bass_kernel_spmd

    nc = _get_program(debug)
    return run_bass_kernel_spmd(nc, in_maps, list(range(len(in_maps))), trace=trace, **kw)


def kernel(x, Wq, bq, Wk, Wv, bv, Wo, bo):
    x = np.asarray(x, dtype=np.float32)
    assert x.shape == (B, T, D), x.shape
    shared = _prep_shared(
        np.asarray(Wq, np.float32), np.asarray(bq, np.float32),
        np.asarray(Wk, np.float32), np.asarray(Wv, np.float32),
        np.asarray(bv, np.float32), np.asarray(Wo, np.float32),
        np.asarray(bo, np.float32),
    )
    in_maps = []
    for b in range(B):
        m = dict(shared)
        m.update(_prep_x(x[b]))
        in_maps.append(m)

    res = _run(in_maps)
    out = np.empty((B, T, D), np.float32)
    for b in range(B):
        yT = np.asarray(res.results[b]["yT"]).astype(np.float32)  # [P, DC, T]
        out[b] = yT.transpose(1, 0, 2).reshape(D, T).T
    return out


# revision 29
# speedup vs baseline: 1.0115x; 1.0025x over previous
"""Linear attention (silu+1 feature map) MultiHeadAttention for 8x TRN2.

Sharding: data-parallel over batch (B=8 -> 1 batch element per NeuronCore).

Math per core (T=4096, D=1024, H=16, Dh=64), with phi(z) = 1 + s(z),
s(z) = silu(z). Write s_q = silu(scale*q), s_k = silu(scale*k). Then

  kv_h   = phi_k_h^T v_h
         = colsum_v_h                      (rank-1 in e; exact, bf16/fp32)
         + (s_k^T x)_h @ Wv_h^T           (fp8 "G path": replaces v proj)
         + bv_h (x) rowsum(s_k)_h         (rank-1 correction)
  M      = kv^T-blocks @ Wo^T             (block-diag, bf16)
  y^T    = M8^T @ s_q + colsum_M + bo     (fp8; the +1 of phi_q is folded
                                           into colsum_M = ones^T M)

All big GEMMs (q proj, k proj, G = s_k^T x, phase-2) run as fp8-e4m3
DoubleRow matmuls (2x128-row contraction @ 0.5 cyc/row).  Centering the
+1 out of phi keeps fp8 noise confined to the ~12%-magnitude fluctuation
terms; exact colsums are carried in fp32/bf16.  Weights are scaled by 64
before fp8 quantization to clear the e4m3 subnormal floor; the inverse
scale rides the ACT silu drain.
"""

import numpy as np
import ml_dtypes

B, T, D = 8, 4096, 1024
H, DH = 16, 64
SCALE = float(DH ** -0.25)
NCORES = 8
P = 128
DC = D // P            # 8 feature chunks
NG = T // 256          # 16 groups of 256 tokens
WS = 64.0              # fp8 weight prescale

_BF16 = ml_dtypes.bfloat16
_F8 = ml_dtypes.float8_e4m3

_CACHE = {}


def _split_multi_waits(nc):
    """walrus in this container only encodes ONE sync-wait command per
    instruction. Hoist extra waits onto injected same-engine NOPs placed
    immediately before the instruction."""
    import concourse.mybir as mybir

    n_split = 0
    for fn in nc.m.functions:
        for bb in fn.blocks:
            new = []
            changed = False
            for inst in bb.instructions:
                si = inst.sync_info
                waits = list(si.on_wait) if si is not None else []
                if len(waits) > 1:
                    changed = True
                    for j, w in enumerate(waits[:-1]):
                        nop = mybir.InstNoOp(
                            name=f"{inst.name}-sw{j}", ins=[], outs=[]
                        )
                        nop.engine = inst.engine
                        nop.sync_info = mybir.SyncInfo(
                            on_wait=[w], on_update=[]
                        )
                        new.append(nop)
                        n_split += 1
                    inst.sync_info = mybir.SyncInfo(
                        on_wait=[waits[-1]], on_update=list(si.on_update)
                    )
                new.append(inst)
            if changed:
                bb.instructions = new
    return n_split


def _build_program(debug=False):
    import concourse.bass as bass
    import concourse.mybir as mybir
    from concourse.tile import TileContext

    dt = mybir.dt
    AF = mybir.ActivationFunctionType
    DR = mybir.MatmulPerfMode.DoubleRow

    nc = bass.Bass()

    xT8_d = nc.dram_tensor("xT8", [P, DC, T], dt.float8e4, kind="ExternalInput")
    xt8_d = nc.dram_tensor("xt8", [P, NG, 2, D], dt.float8e4, kind="ExternalInput")
    xr8_d = nc.dram_tensor("xr8", [P, NG, 2, D], dt.float8e4, kind="ExternalInput")
    wq8_d = nc.dram_tensor("wq8", [P, DC, D], dt.float8e4, kind="ExternalInput")
    wk8_d = nc.dram_tensor("wk8", [P, DC, D], dt.float8e4, kind="ExternalInput")
    wvT_d = nc.dram_tensor("wvT", [P, DC, D], dt.bfloat16, kind="ExternalInput")
    woT_d = nc.dram_tensor("woT", [P, DC, D], dt.bfloat16, kind="ExternalInput")
    bqs_d = nc.dram_tensor("bqs", [P, DC], dt.float32, kind="ExternalInput")
    bqa_d = nc.dram_tensor("bqa", [P, DC], dt.float32, kind="ExternalInput")
    bvc_d = nc.dram_tensor("bvc", [P, DC], dt.float32, kind="ExternalInput")
    bob_d = nc.dram_tensor("bob", [P, DC], dt.float32, kind="ExternalInput")
    bvr_d = nc.dram_tensor("bvr", [1, D], dt.bfloat16, kind="ExternalInput")
    one8_d = nc.dram_tensor("one8", [P, 2, 16], dt.float8e4, kind="ExternalInput")
    yT_d = nc.dram_tensor("yT", [P, DC, T], dt.bfloat16, kind="ExternalOutput")
    if debug:
        dbg = {
            "sq": nc.dram_tensor("dbg_sq", [P, DC, T], dt.float8e4, kind="ExternalOutput"),
            "sk": nc.dram_tensor("dbg_sk", [P, NG, 2, D], dt.float8e4, kind="ExternalOutput"),
            "gt": nc.dram_tensor("dbg_gt", [P, DC, D], dt.bfloat16, kind="ExternalOutput"),
            "kv": nc.dram_tensor("dbg_kv", [P, DC, P], dt.bfloat16, kind="ExternalOutput"),
            "m8": nc.dram_tensor("dbg_m8", [P, DC, D], dt.float8e4, kind="ExternalOutput"),
            "csx": nc.dram_tensor("dbg_csx", [P, DC], dt.bfloat16, kind="ExternalOutput"),
            "cv": nc.dram_tensor("dbg_cv", [P, DC], dt.float32, kind="ExternalOutput"),
            "rs": nc.dram_tensor("dbg_rs", [1, D], dt.bfloat16, kind="ExternalOutput"),
            "by": nc.dram_tensor("dbg_by", [P, DC], dt.float32, kind="ExternalOutput"),
        }

    with TileContext(nc) as tc:
      with tc.tile_pool(name="persist", bufs=1) as pp:
        bqs_sb = pp.tile([P, DC], dt.float32, tag="bqs")
        bqa_sb = pp.tile([P, DC], dt.float32, tag="bqa")
        bvc_sb = pp.tile([P, DC], dt.float32, tag="bvc")
        bob_sb = pp.tile([P, DC], dt.float32, tag="bob")
        bvr_sb = pp.tile([1, D], dt.bfloat16, tag="bvr")
        one8_sb = pp.tile([P, 2, 16], dt.float8e4, tag="one8")
        zz = pp.tile([1, 640], dt.bfloat16, tag="zz")
        csxb_sb = pp.tile([P, DC], dt.bfloat16, tag="csxb")
        cv_sb = pp.tile([P, DC], dt.float32, tag="cv")
        u_sb = pp.tile([P, DC], dt.float32, tag="u")
        ub_sb = pp.tile([P, DC], dt.bfloat16, tag="ub")
        by_sb = pp.tile([P, DC], dt.float32, tag="by")
        rs_sb = pp.tile([1, D], dt.bfloat16, tag="rs")
        kvch = pp.tile([P, DC, P], dt.bfloat16, tag="kvch")
        m8_sb = pp.tile([P, DC, D], dt.float8e4, tag="m8")
        sq_sb = pp.tile([P, DC, T], dt.float8e4, tag="sq")

        nc.vector.memset(zz[:1, 0:256], 0.0)
        nc.vector.memset(kvch[:], 0.0)

        with tc.tile_pool(name="bigB", bufs=1) as pb:
          sk_sb = pb.tile([P, NG, 2, D], dt.float8e4, tag="sk")
          xt8_sb = pb.tile([P, NG, 2, D], dt.float8e4, tag="xt8")
          gt_sb = pb.tile([P, DC, D], dt.bfloat16, tag="gt")

          with tc.tile_pool(name="ph1w", bufs=1) as pc:
            wq8_sb = pc.tile([P, DC, D], dt.float8e4, tag="wq8")
            wk8_sb = pc.tile([P, DC, D], dt.float8e4, tag="wk8")
            xT8_sb = pc.tile([P, DC, T], dt.float8e4, tag="xT8")

            # ============ phase 1a (k proj) + colsum-x, xr8 scoped ============
            with tc.tile_pool(name="ph1r", bufs=1) as pcr:
                xr8_sb = pcr.tile([P, NG, 2, D], dt.float8e4, tag="xr8")

                # one DMA queue (gpsimd: 25ns/trigger), strict priority order
                nc.gpsimd.dma_start(wk8_sb[:], wk8_d[:])
                nc.gpsimd.dma_start(xT8_sb[:, :, 0:512], xT8_d[:, :, 0:512])
                nc.gpsimd.dma_start(xT8_sb[:, :, 512:1024], xT8_d[:, :, 512:1024])
                nc.gpsimd.dma_start(xT8_sb[:, :, 1024:2048], xT8_d[:, :, 1024:2048])
                nc.gpsimd.dma_start(wq8_sb[:], wq8_d[:])
                nc.gpsimd.dma_start(xT8_sb[:, :, 2048:3072], xT8_d[:, :, 2048:3072])
                nc.gpsimd.dma_start(xT8_sb[:, :, 3072:4096], xT8_d[:, :, 3072:4096])
                nc.gpsimd.dma_start(bqs_sb[:], bqs_d[:])
                nc.gpsimd.dma_start(bqa_sb[:], bqa_d[:])
                nc.gpsimd.dma_start(one8_sb[:], one8_d[:])
                nc.gpsimd.dma_start(bvc_sb[:], bvc_d[:])
                nc.gpsimd.dma_start(bob_sb[:], bob_d[:])
                nc.gpsimd.dma_start(bvr_sb[:], bvr_d[:])
                for qq in range(4):
                    nc.gpsimd.dma_start(
                        xt8_sb[:, qq * 4 : (qq + 1) * 4, :, :],
                        xt8_d[:, qq * 4 : (qq + 1) * 4, :, :],
                    )
                for hh in range(2):
                    nc.gpsimd.dma_start(
                        xr8_sb[:, hh * 8 : (hh + 1) * 8, :, :],
                        xr8_d[:, hh * 8 : (hh + 1) * 8, :, :],
                    )

                with (
                    tc.tile_pool(name="kps", bufs=2, space="PSUM") as kpool,
                    tc.tile_pool(name="csps", bufs=1, space="PSUM") as cspool,
                    tc.tile_pool(name="rsps", bufs=1, space="PSUM") as rspool,
                ):
                    csp = cspool.tile([P, DC], dt.float32, tag="csp")
                    rsp = rspool.tile([1, D], dt.float32, tag="rsp")
                    warm = kpool.tile([P, D], dt.float32, tag="kp")
                    for w in range(78):
                        nc.tensor.matmul(
                            warm[:, 0:128], lhsT=zz[:1, :P],
                            rhs=zz[:1, P : P + 128],
                            start=True, stop=True, skip_group_check=True,
                        )

                    def _k_group(g):
                        for i in range(2):
                            kp = kpool.tile([P, D], dt.float32, tag="kp")
                            t0 = g * 256 + i * 128
                            for ds in range(4):
                                for c in range(4):
                                    nc.tensor.matmul(
                                        kp[:, ds * 256 : (ds + 1) * 256],
                                        lhsT=xT8_sb[:, 2 * c : 2 * c + 2, t0 : t0 + 128],
                                        rhs=wk8_sb[:, 2 * c : 2 * c + 2, ds * 256 : (ds + 1) * 256],
                                        start=(c == 0), stop=(c == 3),
                                        perf_mode=DR, skip_group_check=True,
                                    )
                            nc.scalar.activation(
                                sk_sb[:, g, i, :], kp[:], AF.Silu,
                                scale=SCALE / WS,
                            )

                    for g in range(NG):
                        _k_group(g)

                    # colsum_x column: contract tokens against ones; x8 and
                    # the fp8 residual accumulate into one region
                    for cc in range(DC):
                        for g in range(NG):
                            nc.tensor.matmul(
                                csp[:, cc : cc + 1],
                                lhsT=xt8_sb[:, g, :, cc * P : (cc + 1) * P],
                                rhs=one8_sb[:, :, 0:1],
                                start=(g == 0), stop=False,
                                perf_mode=DR, skip_group_check=True,
                            )
                        for g in range(NG):
                            nc.tensor.matmul(
                                csp[:, cc : cc + 1],
                                lhsT=xr8_sb[:, g, :, cc * P : (cc + 1) * P],
                                rhs=one8_sb[:, :, 0:1],
                                start=False, stop=(g == NG - 1),
                                perf_mode=DR, skip_group_check=True,
                            )
                    nc.vector.tensor_copy(out=csxb_sb[:], in_=csp[:])
                    # rowsum(s_k): s_k is complete; PE here idles behind
                    # the ACT silu backlog anyway
                    for ds in range(4):
                        for g in range(NG):
                            nc.tensor.matmul(
                                rsp[:, ds * 256 : (ds + 1) * 256],
                                lhsT=one8_sb[:, :, 0:1],
                                rhs=sk_sb[:, g, :, ds * 256 : (ds + 1) * 256],
                                start=(g == 0), stop=(g == NG - 1),
                                perf_mode=DR, skip_group_check=True,
                            )
                    nc.scalar.copy(out=rs_sb[:], in_=rsp[:])

            # ====== xr8 space free: load wvT/woT during 1b ======
            with tc.tile_pool(name="postw", bufs=1) as pd:
                wvT_sb = pd.tile([P, DC, D], dt.bfloat16, tag="wvT")
                woT_sb = pd.tile([P, DC, D], dt.bfloat16, tag="woT")
                nc.sync.dma_start(wvT_sb[:], wvT_d[:])
                nc.sync.dma_start(woT_sb[:], woT_d[:])

                # ====== phase 1b: q proj interleaved with G half-chunks ======
                with (
                    tc.tile_pool(name="qps", bufs=4, space="PSUM") as qpool,
                    tc.tile_pool(name="gps", bufs=2, space="PSUM") as gpool,
                ):
                    def _q_group(g):
                        for half in range(4):
                            oc0 = half * 2
                            qp = qpool.tile([P, 2, 256], dt.float32, tag="qp")
                            for j in range(2):
                                oc = oc0 + j
                                for c in range(4):
                                    nc.tensor.matmul(
                                        qp[:, j, :],
                                        lhsT=wq8_sb[:, 2 * c : 2 * c + 2, oc * P : (oc + 1) * P],
                                        rhs=xT8_sb[:, 2 * c : 2 * c + 2, g * 256 : (g + 1) * 256],
                                        start=(c == 0), stop=(c == 3),
                                        perf_mode=DR, skip_group_check=True,
                                    )
                            dst = sq_sb[:, oc0 : oc0 + 2, g * 256 : (g + 1) * 256]
                            if half % 2 == 0:
                                # bias fused into ACT silu, per oc
                                for j in range(2):
                                    nc.scalar.activation(
                                        sq_sb[:, oc0 + j, g * 256 : (g + 1) * 256],
                                        qp[:, j, :], AF.Silu,
                                        bias=bqa_sb[:, oc0 + j : oc0 + j + 1],
                                        scale=SCALE / WS,
                                    )
                            else:
                                # bias on DVE (WS-scaled), plain silu on ACT
                                for j in range(2):
                                    nc.vector.tensor_scalar_add(
                                        qp[:, j, :], qp[:, j, :],
                                        bqs_sb[:, oc0 + j : oc0 + j + 1],
                                    )
                                nc.scalar.activation(
                                    dst, qp[:], AF.Silu, scale=SCALE / WS,
                                )

                    g_tiles = {}

                    def _g_half(idx):
                        cc, half = idx // 2, idx % 2
                        if half == 0:
                            gp = gpool.tile([P, D], dt.float32, tag="gp")
                            g_tiles[cc] = gp
                        else:
                            gp = g_tiles[cc]
                        # ds-serial regions: each region's g==0 matmul carries
                        # start=True; a later region's start only re-marks the
                        # bank's zero-flags, which no finished region rewrites
                        for ds in (half * 2, half * 2 + 1):
                            for g in range(NG):
                                nc.tensor.matmul(
                                    gp[:, ds * 256 : (ds + 1) * 256],
                                    lhsT=xt8_sb[:, g, :, cc * P : (cc + 1) * P],
                                    rhs=sk_sb[:, g, :, ds * 256 : (ds + 1) * 256],
                                    start=(g == 0), stop=(g == NG - 1),
                                    perf_mode=DR, skip_group_check=True,
                                )
                        if half == 1:
                            nc.vector.tensor_copy(out=gt_sb[:, cc, :], in_=gp[:])

                    for g in range(NG):
                        _q_group(g)
                        _g_half(g)

                # ---- rowsum(s_k), colsum_v, kv blocks (one psum pool:
                # no pool-exit WAR between them) ----
                with tc.tile_pool(name="rkv", bufs=1, space="PSUM") as rkpool:
                    cvp = rkpool.tile([P, DC], dt.float32, tag="cvp")
                    # colsum_v = Wv @ colsum_x  (+ T*bv via bvc)
                    for b in range(DC):
                        for cc in range(DC):
                            nc.tensor.matmul(
                                cvp[:, b : b + 1],
                                lhsT=wvT_sb[:, cc, b * P : (b + 1) * P],
                                rhs=csxb_sb[:, cc : cc + 1],
                                start=(cc == 0), stop=(cc == DC - 1),
                                skip_group_check=True,
                            )
                    nc.vector.tensor_add(cv_sb[:], cvp[:], bvc_sb[:])

                # kv blocks: one PSUM bank per block (start=True zeroes a
                # whole 2KB bank region, so sharing banks WAR-serializes on
                # drains)
                with tc.tile_pool(name="kvbk", bufs=1, space="PSUM") as kvpool:
                    for b in range(DC):
                        kvp = kvpool.tile([P, 512], dt.float32,
                                          tag=f"kvp{b}", name=f"kvp{b}")
                        for cc in range(DC):
                            nc.tensor.matmul(
                                kvp[:, 0:P],
                                lhsT=wvT_sb[:, cc, b * P : (b + 1) * P],
                                rhs=gt_sb[:, cc, b * P : (b + 1) * P],
                                start=(cc == 0), stop=False,
                                skip_group_check=True,
                            )
                        nc.tensor.matmul(
                            kvp[:, 0:P],
                            lhsT=bvr_sb[:1, b * P : (b + 1) * P],
                            rhs=rs_sb[:1, b * P : (b + 1) * P],
                            start=False, stop=True, skip_group_check=True,
                        )
                        nc.scalar.activation(
                            kvch[0:64, b, 0:64], kvp[0:64, 0:64],
                            AF.Identity, bias=cv_sb[0:64, b : b + 1],
                        )
                        nc.scalar.activation(
                            kvch[64:128, b, 64:128], kvp[64:128, 64:128],
                            AF.Identity, bias=cv_sb[64:128, b : b + 1],
                        )
                        nc.vector.tensor_reduce(
                            u_sb[0:64, b : b + 1], kvp[0:64, 0:64],
                            axis=mybir.AxisListType.X, op=mybir.AluOpType.add,
                        )
                        nc.vector.tensor_reduce(
                            u_sb[64:128, b : b + 1], kvp[64:128, 64:128],
                            axis=mybir.AxisListType.X, op=mybir.AluOpType.add,
                        )
                    # u includes the cv bias once per within-head column
                    nc.vector.scalar_tensor_tensor(
                        out=ub_sb[:], in0=cv_sb[:], scalar=float(DH),
                        in1=u_sb[:], op0=mybir.AluOpType.mult,
                        op1=mybir.AluOpType.add,
                    )

                # M = kv^T @ Wo^T ; colsum_M
                with (
                    tc.tile_pool(name="mps", bufs=3, space="PSUM") as mpool,
                    tc.tile_pool(name="cmps", bufs=1, space="PSUM") as cmpool,
                ):
                    for b in range(DC):
                        mp = mpool.tile([P, D], dt.float32, tag="mp")
                        for hh in range(2):
                            nc.tensor.matmul(
                                mp[:, hh * 512 : (hh + 1) * 512],
                                lhsT=kvch[:, b, :],
                                rhs=woT_sb[:, b, hh * 512 : (hh + 1) * 512],
                                start=True, stop=True, skip_group_check=True,
                            )
                        if b % 2 == 0:
                            nc.scalar.copy(out=m8_sb[:, b, :], in_=mp[:])
                        else:
                            nc.vector.tensor_copy(out=m8_sb[:, b, :], in_=mp[:])
                    cmp_t = cmpool.tile([P, DC], dt.float32, tag="cmp")
                    for oc in range(DC):
                        for b in range(DC):
                            nc.tensor.matmul(
                                cmp_t[:, oc : oc + 1],
                                lhsT=woT_sb[:, b, oc * P : (oc + 1) * P],
                                rhs=ub_sb[:, b : b + 1],
                                start=(b == 0), stop=(b == DC - 1),
                                skip_group_check=True,
                            )
                    nc.vector.tensor_add(by_sb[:], cmp_t[:], bob_sb[:])

                if debug:
                    nc.sync.dma_start(dbg["sq"][:], sq_sb[:])
                    nc.sync.dma_start(dbg["sk"][:], sk_sb[:])
                    nc.sync.dma_start(dbg["gt"][:], gt_sb[:])
                    nc.sync.dma_start(dbg["kv"][:], kvch[:])
                    nc.sync.dma_start(dbg["m8"][:], m8_sb[:])
                    nc.sync.dma_start(dbg["csx"][:], csxb_sb[:])
                    nc.sync.dma_start(dbg["cv"][:], cv_sb[:])
                    nc.sync.dma_start(dbg["rs"][:], rs_sb[:])
                    nc.sync.dma_start(dbg["by"][:], by_sb[:])

        # ================= phase 2: y^T = M8^T s_q + bias =================
        with (
            tc.tile_pool(name="yout", bufs=6) as ypool,
            tc.tile_pool(name="yps", bufs=8, space="PSUM") as ypsp,
        ):
            n = 0
            for oc in range(DC):
                for tp in range(8):
                    if tp % 2 == 0:
                        ys = ypool.tile(
                            [P, 2, 512], dt.bfloat16, tag="ys", name="ys"
                        )
                    yp = ypsp.tile([P, 512], dt.float32, tag="yp")
                    for hh in range(2):
                        ts = tp * 2 + hh
                        for f in range(4):
                            nc.tensor.matmul(
                                yp[:, hh * 256 : (hh + 1) * 256],
                                lhsT=m8_sb[:, 2 * f : 2 * f + 2, oc * P : (oc + 1) * P],
                                rhs=sq_sb[:, 2 * f : 2 * f + 2, ts * 256 : (ts + 1) * 256],
                                start=(f == 0), stop=(f == 3),
                                perf_mode=DR, skip_group_check=True,
                            )
                    if n % 2 == 0:
                        nc.scalar.activation(
                            ys[:, tp % 2, :], yp[:], AF.Identity,
                            bias=by_sb[:, oc : oc + 1], scale=1.0,
                        )
                    else:
                        nc.vector.tensor_scalar_add(
                            ys[:, tp % 2, :], yp[:], by_sb[:, oc : oc + 1]
                        )
                    if tp % 2 == 1:
                        # keep the last transfers on sync (HWDGE beats the
                        # Pool SWDGE's 1us desc-gen at the kernel tail)
                        q = nc.sync if ((n // 2) % 2 == 0 or n >= 56) \
                            else nc.gpsimd
                        q.dma_start(
                            yT_d[:, oc, (tp - 1) * 512 : (tp + 1) * 512],
                            ys[:],
                        )
                    n += 1

    _split_multi_waits(nc)
    return nc


def _get_program(debug=False):
    key = ("nc", debug)
    if key not in _CACHE:
        _CACHE[key] = _build_program(debug)
    return _CACHE[key]


def _prep_shared(Wq, bq, Wk, Wv, bv, Wo, bo):
    def wchunk(w, dtype, scale=1.0):
        # [D, D] row-major (contract, out) -> [P, DC, D] with c = cc*128+p
        return np.ascontiguousarray(
            (w * scale).T.reshape(DC, P, D).transpose(1, 0, 2)
        ).astype(dtype)

    shared = {
        "wq8": wchunk(Wq, _F8, WS),
        "wk8": wchunk(Wk, _F8, WS),
        "wvT": wchunk(Wv, _BF16),
        "woT": wchunk(Wo, _BF16),
        # DVE/ACT pre-add this to the WS-scaled q PSUM; ACT then multiplies
        # by SCALE/WS, so the bias carries WS (not SCALE).
        "bqs": np.ascontiguousarray((WS * bq).astype(np.float32).reshape(DC, P).T),
        "bqa": np.ascontiguousarray((SCALE * bq).astype(np.float32).reshape(DC, P).T),
        "bvc": np.ascontiguousarray((T * bv).astype(np.float32).reshape(DC, P).T),
        "bob": np.ascontiguousarray(bo.astype(np.float32).reshape(DC, P).T),
        "bvr": bv.astype(_BF16)[None, :],
        "one8": np.ones((P, 2, 16), _F8),
    }
    return shared


def _prep_x(xb):
    xT = np.ascontiguousarray(xb.T)  # [D, T]
    x8 = xb.astype(_F8)
    xr8 = (xb - x8.astype(np.float32)).astype(_F8)

    def tok(a):
        return np.ascontiguousarray(
            a.reshape(NG, 2, P, D).transpose(2, 0, 1, 3)
        )

    return {
        "xT8": np.ascontiguousarray(
            xT.reshape(DC, P, T).transpose(1, 0, 2)
        ).astype(_F8),
        "xt8": tok(x8),
        "xr8": tok(xr8),
    }


def _run(in_maps, trace=False, debug=False, **kw):
    from concourse.bass_utils import run_bass_kernel_spmd

    nc = _get_program(debug)
    return run_bass_kernel_spmd(nc, in_maps, list(range(len(in_maps))), trace=trace, **kw)


def kernel(x, Wq, bq, Wk, Wv, bv, Wo, bo):
    x = np.asarray(x, dtype=np.float32)
    assert x.shape == (B, T, D), x.shape
    shared = _prep_shared(
        np.asarray(Wq, np.float32), np.asarray(bq, np.float32),
        np.asarray(Wk, np.float32), np.asarray(Wv, np.float32),
        np.asarray(bv, np.float32), np.asarray(Wo, np.float32),
        np.asarray(bo, np.float32),
    )
    in_maps = []
    for b in range(B):
        m = dict(shared)
        m.update(_prep_x(x[b]))
        in_maps.append(m)

    res = _run(in_maps)
    out = np.empty((B, T, D), np.float32)
    for b in range(B):
        yT = np.asarray(res.results[b]["yT"]).astype(np.float32)  # [P, DC, T]
        out[b] = yT.transpose(1, 0, 2).reshape(D, T).T
    return out


# revision 31
# speedup vs baseline: 1.0236x; 1.0120x over previous
"""Linear attention (silu+1 feature map) MultiHeadAttention for 8x TRN2.

Sharding: data-parallel over batch (B=8 -> 1 batch element per NeuronCore).

Math per core (T=4096, D=1024, H=16, Dh=64), with phi(z) = 1 + s(z),
s(z) = silu(z). Write s_q = silu(scale*q), s_k = silu(scale*k). Then

  kv_h   = phi_k_h^T v_h
         = colsum_v_h                      (rank-1 in e; exact, bf16/fp32)
         + (s_k^T x)_h @ Wv_h^T           (fp8 "G path": replaces v proj)
         + bv_h (x) rowsum(s_k)_h         (rank-1 correction)
  M      = kv^T-blocks @ Wo^T             (block-diag, bf16)
  y^T    = M8^T @ s_q + colsum_M + bo     (fp8; the +1 of phi_q is folded
                                           into colsum_M = ones^T M)

All big GEMMs (q proj, k proj, G = s_k^T x, phase-2) run as fp8-e4m3
DoubleRow matmuls (2x128-row contraction @ 0.5 cyc/row).  Centering the
+1 out of phi keeps fp8 noise confined to the ~12%-magnitude fluctuation
terms; exact colsums are carried in fp32/bf16.  Weights are scaled by 64
before fp8 quantization to clear the e4m3 subnormal floor; the inverse
scale rides the ACT silu drain.
"""

import numpy as np
import ml_dtypes

B, T, D = 8, 4096, 1024
H, DH = 16, 64
SCALE = float(DH ** -0.25)
NCORES = 8
P = 128
DC = D // P            # 8 feature chunks
NG = T // 256          # 16 groups of 256 tokens
WS = 64.0              # fp8 weight prescale

_BF16 = ml_dtypes.bfloat16
_F8 = ml_dtypes.float8_e4m3

_CACHE = {}


def _split_multi_waits(nc):
    """walrus in this container only encodes ONE sync-wait command per
    instruction. Hoist extra waits onto injected same-engine NOPs placed
    immediately before the instruction."""
    import concourse.mybir as mybir

    n_split = 0
    for fn in nc.m.functions:
        for bb in fn.blocks:
            new = []
            changed = False
            for inst in bb.instructions:
                si = inst.sync_info
                waits = list(si.on_wait) if si is not None else []
                if len(waits) > 1:
                    changed = True
                    for j, w in enumerate(waits[:-1]):
                        nop = mybir.InstNoOp(
                            name=f"{inst.name}-sw{j}", ins=[], outs=[]
                        )
                        nop.engine = inst.engine
                        nop.sync_info = mybir.SyncInfo(
                            on_wait=[w], on_update=[]
                        )
                        new.append(nop)
                        n_split += 1
                    inst.sync_info = mybir.SyncInfo(
                        on_wait=[waits[-1]], on_update=list(si.on_update)
                    )
                new.append(inst)
            if changed:
                bb.instructions = new
    return n_split


def _build_program(debug=False):
    import concourse.bass as bass
    import concourse.mybir as mybir
    from concourse.tile import TileContext

    dt = mybir.dt
    AF = mybir.ActivationFunctionType
    DR = mybir.MatmulPerfMode.DoubleRow

    nc = bass.Bass()

    xT8_d = nc.dram_tensor("xT8", [P, DC, T], dt.float8e4, kind="ExternalInput")
    xt8_d = nc.dram_tensor("xt8", [P, NG, 2, D], dt.float8e4, kind="ExternalInput")
    xr8_d = nc.dram_tensor("xr8", [P, NG, 2, D], dt.float8e4, kind="ExternalInput")
    wq8_d = nc.dram_tensor("wq8", [P, DC, D], dt.float8e4, kind="ExternalInput")
    wk8_d = nc.dram_tensor("wk8", [P, DC, D], dt.float8e4, kind="ExternalInput")
    wvT_d = nc.dram_tensor("wvT", [P, DC, D], dt.bfloat16, kind="ExternalInput")
    woT_d = nc.dram_tensor("woT", [P, DC, D], dt.bfloat16, kind="ExternalInput")
    bqs_d = nc.dram_tensor("bqs", [P, DC], dt.float32, kind="ExternalInput")
    bqa_d = nc.dram_tensor("bqa", [P, DC], dt.float32, kind="ExternalInput")
    bvc_d = nc.dram_tensor("bvc", [P, DC], dt.float32, kind="ExternalInput")
    bob_d = nc.dram_tensor("bob", [P, DC], dt.float32, kind="ExternalInput")
    bvr_d = nc.dram_tensor("bvr", [1, D], dt.bfloat16, kind="ExternalInput")
    one8_d = nc.dram_tensor("one8", [P, 2, 16], dt.float8e4, kind="ExternalInput")
    yT_d = nc.dram_tensor("yT", [P, DC, T], dt.bfloat16, kind="ExternalOutput")
    if debug:
        dbg = {
            "sq": nc.dram_tensor("dbg_sq", [P, DC, T], dt.float8e4, kind="ExternalOutput"),
            "sk": nc.dram_tensor("dbg_sk", [P, NG, 2, D], dt.float8e4, kind="ExternalOutput"),
            "gt": nc.dram_tensor("dbg_gt", [P, DC, D], dt.bfloat16, kind="ExternalOutput"),
            "kv": nc.dram_tensor("dbg_kv", [P, DC, P], dt.bfloat16, kind="ExternalOutput"),
            "m8": nc.dram_tensor("dbg_m8", [P, DC, D], dt.float8e4, kind="ExternalOutput"),
            "csx": nc.dram_tensor("dbg_csx", [P, DC], dt.bfloat16, kind="ExternalOutput"),
            "cv": nc.dram_tensor("dbg_cv", [P, DC], dt.float32, kind="ExternalOutput"),
            "rs": nc.dram_tensor("dbg_rs", [1, D], dt.bfloat16, kind="ExternalOutput"),
            "by": nc.dram_tensor("dbg_by", [P, DC], dt.float32, kind="ExternalOutput"),
        }

    with TileContext(nc) as tc:
      with tc.tile_pool(name="persist", bufs=1) as pp:
        bqs_sb = pp.tile([P, DC], dt.float32, tag="bqs")
        bqa_sb = pp.tile([P, DC], dt.float32, tag="bqa")
        bvc_sb = pp.tile([P, DC], dt.float32, tag="bvc")
        bob_sb = pp.tile([P, DC], dt.float32, tag="bob")
        bvr_sb = pp.tile([1, D], dt.bfloat16, tag="bvr")
        one8_sb = pp.tile([P, 2, 16], dt.float8e4, tag="one8")
        zz = pp.tile([1, 640], dt.bfloat16, tag="zz")
        csxb_sb = pp.tile([P, DC], dt.bfloat16, tag="csxb")
        cv_sb = pp.tile([P, DC], dt.float32, tag="cv")
        u_sb = pp.tile([P, DC], dt.float32, tag="u")
        ub_sb = pp.tile([P, DC], dt.bfloat16, tag="ub")
        by_sb = pp.tile([P, DC], dt.float32, tag="by")
        rs_sb = pp.tile([1, D], dt.bfloat16, tag="rs")
        kvch = pp.tile([P, DC, P], dt.bfloat16, tag="kvch")
        m8_sb = pp.tile([P, DC, D], dt.float8e4, tag="m8")
        sq_sb = pp.tile([P, DC, T], dt.float8e4, tag="sq")

        nc.vector.memset(zz[:1, 0:256], 0.0)
        nc.vector.memset(kvch[:], 0.0)

        with tc.tile_pool(name="bigB", bufs=1) as pb:
          sk_sb = pb.tile([P, NG, 2, D], dt.float8e4, tag="sk")
          xt8_sb = pb.tile([P, NG, 2, D], dt.float8e4, tag="xt8")
          gt_sb = pb.tile([P, DC, D], dt.bfloat16, tag="gt")

          with tc.tile_pool(name="ph1w", bufs=1) as pc:
            wq8_sb = pc.tile([P, DC, D], dt.float8e4, tag="wq8")
            wk8_sb = pc.tile([P, DC, D], dt.float8e4, tag="wk8")
            xT8_sb = pc.tile([P, DC, T], dt.float8e4, tag="xT8")

            # ============ phase 1a (k proj) + colsum-x, xr8 scoped ============
            with tc.tile_pool(name="ph1r", bufs=1) as pcr:
                xr8_sb = pcr.tile([P, NG, 2, D], dt.float8e4, tag="xr8")

                # one DMA queue (gpsimd: 25ns/trigger), strict priority order
                nc.gpsimd.dma_start(wk8_sb[:], wk8_d[:])
                nc.gpsimd.dma_start(xT8_sb[:, :, 0:512], xT8_d[:, :, 0:512])
                nc.gpsimd.dma_start(xT8_sb[:, :, 512:1024], xT8_d[:, :, 512:1024])
                nc.gpsimd.dma_start(xT8_sb[:, :, 1024:2048], xT8_d[:, :, 1024:2048])
                nc.gpsimd.dma_start(wq8_sb[:], wq8_d[:])
                nc.gpsimd.dma_start(xT8_sb[:, :, 2048:3072], xT8_d[:, :, 2048:3072])
                nc.gpsimd.dma_start(xT8_sb[:, :, 3072:4096], xT8_d[:, :, 3072:4096])
                nc.gpsimd.dma_start(bqs_sb[:], bqs_d[:])
                nc.gpsimd.dma_start(bqa_sb[:], bqa_d[:])
                nc.gpsimd.dma_start(one8_sb[:], one8_d[:])
                nc.gpsimd.dma_start(bvc_sb[:], bvc_d[:])
                nc.gpsimd.dma_start(bob_sb[:], bob_d[:])
                nc.gpsimd.dma_start(bvr_sb[:], bvr_d[:])
                for qq in range(4):
                    nc.gpsimd.dma_start(
                        xt8_sb[:, qq * 4 : (qq + 1) * 4, :, :],
                        xt8_d[:, qq * 4 : (qq + 1) * 4, :, :],
                    )
                for hh in range(2):
                    nc.gpsimd.dma_start(
                        xr8_sb[:, hh * 8 : (hh + 1) * 8, :, :],
                        xr8_d[:, hh * 8 : (hh + 1) * 8, :, :],
                    )

                with (
                    tc.tile_pool(name="kps", bufs=3, space="PSUM") as kpool,
                    tc.tile_pool(name="csps", bufs=1, space="PSUM") as cspool,
                ):
                    csp = cspool.tile([P, DC], dt.float32, tag="csp")
                    warm = kpool.tile([P, D], dt.float32, tag="kp")
                    for w in range(78):
                        nc.tensor.matmul(
                            warm[:, 0:128], lhsT=zz[:1, :P],
                            rhs=zz[:1, P : P + 128],
                            start=True, stop=True, skip_group_check=True,
                        )

                    def _k_group(g):
                        for i in range(2):
                            kp = kpool.tile([P, D], dt.float32, tag="kp")
                            t0 = g * 256 + i * 128
                            for ds in range(4):
                                for c in range(4):
                                    nc.tensor.matmul(
                                        kp[:, ds * 256 : (ds + 1) * 256],
                                        lhsT=xT8_sb[:, 2 * c : 2 * c + 2, t0 : t0 + 128],
                                        rhs=wk8_sb[:, 2 * c : 2 * c + 2, ds * 256 : (ds + 1) * 256],
                                        start=(c == 0), stop=(c == 3),
                                        perf_mode=DR, skip_group_check=True,
                                    )
                            nc.scalar.activation(
                                sk_sb[:, g, i, :], kp[:], AF.Silu,
                                scale=SCALE / WS,
                            )

                    for g in range(NG):
                        _k_group(g)

                    # colsum_x column: contract tokens against ones; x8 and
                    # the fp8 residual accumulate into one region
                    for cc in range(DC):
                        for g in range(NG):
                            nc.tensor.matmul(
                                csp[:, cc : cc + 1],
                                lhsT=xt8_sb[:, g, :, cc * P : (cc + 1) * P],
                                rhs=one8_sb[:, :, 0:1],
                                start=(g == 0), stop=False,
                                perf_mode=DR, skip_group_check=True,
                            )
                        for g in range(NG):
                            nc.tensor.matmul(
                                csp[:, cc : cc + 1],
                                lhsT=xr8_sb[:, g, :, cc * P : (cc + 1) * P],
                                rhs=one8_sb[:, :, 0:1],
                                start=False, stop=(g == NG - 1),
                                perf_mode=DR, skip_group_check=True,
                            )
                    nc.vector.tensor_copy(out=csxb_sb[:], in_=csp[:])
                    # rowsum(s_k): s_k is complete (PE here idles behind the
                    # ACT silu backlog anyway); accumulate into a recycled
                    # k-psum tile -- no extra banks
                    rsp = kpool.tile([P, D], dt.float32, tag="kp", name="rsp")
                    for ds in range(4):
                        for g in range(NG):
                            nc.tensor.matmul(
                                rsp[0:1, ds * 256 : (ds + 1) * 256],
                                lhsT=one8_sb[:, :, 0:1],
                                rhs=sk_sb[:, g, :, ds * 256 : (ds + 1) * 256],
                                start=(g == 0), stop=(g == NG - 1),
                                perf_mode=DR, skip_group_check=True,
                            )
                    nc.scalar.copy(out=rs_sb[:], in_=rsp[0:1, 0:D])

            # ====== xr8 space free: load wvT/woT during 1b ======
            with tc.tile_pool(name="postw", bufs=1) as pd:
                wvT_sb = pd.tile([P, DC, D], dt.bfloat16, tag="wvT")
                woT_sb = pd.tile([P, DC, D], dt.bfloat16, tag="woT")
                nc.sync.dma_start(wvT_sb[:], wvT_d[:])
                nc.sync.dma_start(woT_sb[:], woT_d[:])

                # ====== phase 1b: q proj interleaved with G half-chunks ======
                with (
                    tc.tile_pool(name="qps", bufs=4, space="PSUM") as qpool,
                    tc.tile_pool(name="gps", bufs=2, space="PSUM") as gpool,
                ):
                    def _q_group(g):
                        for half in range(4):
                            oc0 = half * 2
                            qp = qpool.tile([P, 2, 256], dt.float32, tag="qp")
                            for j in range(2):
                                oc = oc0 + j
                                for c in range(4):
                                    nc.tensor.matmul(
                                        qp[:, j, :],
                                        lhsT=wq8_sb[:, 2 * c : 2 * c + 2, oc * P : (oc + 1) * P],
                                        rhs=xT8_sb[:, 2 * c : 2 * c + 2, g * 256 : (g + 1) * 256],
                                        start=(c == 0), stop=(c == 3),
                                        perf_mode=DR, skip_group_check=True,
                                    )
                            dst = sq_sb[:, oc0 : oc0 + 2, g * 256 : (g + 1) * 256]
                            if half % 2 == 0:
                                # bias fused into ACT silu, per oc
                                for j in range(2):
                                    nc.scalar.activation(
                                        sq_sb[:, oc0 + j, g * 256 : (g + 1) * 256],
                                        qp[:, j, :], AF.Silu,
                                        bias=bqa_sb[:, oc0 + j : oc0 + j + 1],
                                        scale=SCALE / WS,
                                    )
                            else:
                                # bias on DVE (WS-scaled), plain silu on ACT
                                for j in range(2):
                                    nc.vector.tensor_scalar_add(
                                        qp[:, j, :], qp[:, j, :],
                                        bqs_sb[:, oc0 + j : oc0 + j + 1],
                                    )
                                nc.scalar.activation(
                                    dst, qp[:], AF.Silu, scale=SCALE / WS,
                                )

                    g_tiles = {}

                    def _g_half(idx):
                        cc, half = idx // 2, idx % 2
                        if half == 0:
                            gp = gpool.tile([P, D], dt.float32, tag="gp")
                            g_tiles[cc] = gp
                        else:
                            gp = g_tiles[cc]
                        # ds-serial regions: each region's g==0 matmul carries
                        # start=True; a later region's start only re-marks the
                        # bank's zero-flags, which no finished region rewrites
                        for ds in (half * 2, half * 2 + 1):
                            for g in range(NG):
                                nc.tensor.matmul(
                                    gp[:, ds * 256 : (ds + 1) * 256],
                                    lhsT=xt8_sb[:, g, :, cc * P : (cc + 1) * P],
                                    rhs=sk_sb[:, g, :, ds * 256 : (ds + 1) * 256],
                                    start=(g == 0), stop=(g == NG - 1),
                                    perf_mode=DR, skip_group_check=True,
                                )
                        if half == 1:
                            nc.vector.tensor_copy(out=gt_sb[:, cc, :], in_=gp[:])

                    for g in range(NG):
                        _q_group(g)
                        _g_half(g)

                # ---- rowsum(s_k), colsum_v, kv blocks (one psum pool:
                # no pool-exit WAR between them) ----
                with tc.tile_pool(name="rkv", bufs=1, space="PSUM") as rkpool:
                    cvp = rkpool.tile([P, DC], dt.float32, tag="cvp")
                    # colsum_v = Wv @ colsum_x  (+ T*bv via bvc)
                    for b in range(DC):
                        for cc in range(DC):
                            nc.tensor.matmul(
                                cvp[:, b : b + 1],
                                lhsT=wvT_sb[:, cc, b * P : (b + 1) * P],
                                rhs=csxb_sb[:, cc : cc + 1],
                                start=(cc == 0), stop=(cc == DC - 1),
                                skip_group_check=True,
                            )
                    nc.vector.tensor_add(cv_sb[:], cvp[:], bvc_sb[:])

                # kv blocks: one PSUM bank per block (start=True zeroes a
                # whole 2KB bank region, so sharing banks WAR-serializes on
                # drains)
                with tc.tile_pool(name="kvbk", bufs=1, space="PSUM") as kvpool:
                    for b in range(DC):
                        kvp = kvpool.tile([P, 512], dt.float32,
                                          tag=f"kvp{b}", name=f"kvp{b}")
                        for cc in range(DC):
                            nc.tensor.matmul(
                                kvp[:, 0:P],
                                lhsT=wvT_sb[:, cc, b * P : (b + 1) * P],
                                rhs=gt_sb[:, cc, b * P : (b + 1) * P],
                                start=(cc == 0), stop=False,
                                skip_group_check=True,
                            )
                        nc.tensor.matmul(
                            kvp[:, 0:P],
                            lhsT=bvr_sb[:1, b * P : (b + 1) * P],
                            rhs=rs_sb[:1, b * P : (b + 1) * P],
                            start=False, stop=True, skip_group_check=True,
                        )
                        nc.scalar.activation(
                            kvch[0:64, b, 0:64], kvp[0:64, 0:64],
                            AF.Identity, bias=cv_sb[0:64, b : b + 1],
                        )
                        nc.scalar.activation(
                            kvch[64:128, b, 64:128], kvp[64:128, 64:128],
                            AF.Identity, bias=cv_sb[64:128, b : b + 1],
                        )
                        nc.vector.tensor_reduce(
                            u_sb[0:64, b : b + 1], kvp[0:64, 0:64],
                            axis=mybir.AxisListType.X, op=mybir.AluOpType.add,
                        )
                        nc.vector.tensor_reduce(
                            u_sb[64:128, b : b + 1], kvp[64:128, 64:128],
                            axis=mybir.AxisListType.X, op=mybir.AluOpType.add,
                        )
                    # u includes the cv bias once per within-head column
                    nc.vector.scalar_tensor_tensor(
                        out=ub_sb[:], in0=cv_sb[:], scalar=float(DH),
                        in1=u_sb[:], op0=mybir.AluOpType.mult,
                        op1=mybir.AluOpType.add,
                    )

                # M = kv^T @ Wo^T ; colsum_M
                with (
                    tc.tile_pool(name="mps", bufs=3, space="PSUM") as mpool,
                    tc.tile_pool(name="cmps", bufs=1, space="PSUM") as cmpool,
                ):
                    for b in range(DC):
                        mp = mpool.tile([P, D], dt.float32, tag="mp")
                        for hh in range(2):
                            nc.tensor.matmul(
                                mp[:, hh * 512 : (hh + 1) * 512],
                                lhsT=kvch[:, b, :],
                                rhs=woT_sb[:, b, hh * 512 : (hh + 1) * 512],
                                start=True, stop=True, skip_group_check=True,
                            )
                        if b % 2 == 0:
                            nc.scalar.copy(out=m8_sb[:, b, :], in_=mp[:])
                        else:
                            nc.vector.tensor_copy(out=m8_sb[:, b, :], in_=mp[:])
                    cmp_t = cmpool.tile([P, DC], dt.float32, tag="cmp")
                    for oc in range(DC):
                        for b in range(DC):
                            nc.tensor.matmul(
                                cmp_t[:, oc : oc + 1],
                                lhsT=woT_sb[:, b, oc * P : (oc + 1) * P],
                                rhs=ub_sb[:, b : b + 1],
                                start=(b == 0), stop=(b == DC - 1),
                                skip_group_check=True,
                            )
                    nc.vector.tensor_add(by_sb[:], cmp_t[:], bob_sb[:])

                if debug:
                    nc.sync.dma_start(dbg["sq"][:], sq_sb[:])
                    nc.sync.dma_start(dbg["sk"][:], sk_sb[:])
                    nc.sync.dma_start(dbg["gt"][:], gt_sb[:])
                    nc.sync.dma_start(dbg["kv"][:], kvch[:])
                    nc.sync.dma_start(dbg["m8"][:], m8_sb[:])
                    nc.sync.dma_start(dbg["csx"][:], csxb_sb[:])
                    nc.sync.dma_start(dbg["cv"][:], cv_sb[:])
                    nc.sync.dma_start(dbg["rs"][:], rs_sb[:])
                    nc.sync.dma_start(dbg["by"][:], by_sb[:])

        # ================= phase 2: y^T = M8^T s_q + bias =================
        with (
            tc.tile_pool(name="yout", bufs=6) as ypool,
            tc.tile_pool(name="yps", bufs=8, space="PSUM") as ypsp,
        ):
            n = 0
            for oc in range(DC):
                for tp in range(8):
                    if tp % 2 == 0:
                        ys = ypool.tile(
                            [P, 2, 512], dt.bfloat16, tag="ys", name="ys"
                        )
                    yp = ypsp.tile([P, 512], dt.float32, tag="yp")
                    for hh in range(2):
                        ts = tp * 2 + hh
                        for f in range(4):
                            nc.tensor.matmul(
                                yp[:, hh * 256 : (hh + 1) * 256],
                                lhsT=m8_sb[:, 2 * f : 2 * f + 2, oc * P : (oc + 1) * P],
                                rhs=sq_sb[:, 2 * f : 2 * f + 2, ts * 256 : (ts + 1) * 256],
                                start=(f == 0), stop=(f == 3),
                                perf_mode=DR, skip_group_check=True,
                            )
                    if n % 2 == 0:
                        nc.scalar.activation(
                            ys[:, tp % 2, :], yp[:], AF.Identity,
                            bias=by_sb[:, oc : oc + 1], scale=1.0,
                        )
                    else:
                        nc.vector.tensor_scalar_add(
                            ys[:, tp % 2, :], yp[:], by_sb[:, oc : oc + 1]
                        )
                    if tp % 2 == 1:
                        # keep the last transfers on sync (HWDGE beats the
                        # Pool SWDGE's 1us desc-gen at the kernel tail)
                        q = nc.sync if ((n // 2) % 2 == 0 or n >= 56) \
                            else nc.gpsimd
                        q.dma_start(
                            yT_d[:, oc, (tp - 1) * 512 : (tp + 1) * 512],
                            ys[:],
                        )
                    n += 1

    _split_multi_waits(nc)
    return nc


def _get_program(debug=False):
    key = ("nc", debug)
    if key not in _CACHE:
        _CACHE[key] = _build_program(debug)
    return _CACHE[key]


def _prep_shared(Wq, bq, Wk, Wv, bv, Wo, bo):
    def wchunk(w, dtype, scale=1.0):
        # [D, D] row-major (contract, out) -> [P, DC, D] with c = cc*128+p
        return np.ascontiguousarray(
            (w * scale).T.reshape(DC, P, D).transpose(1, 0, 2)
        ).astype(dtype)

    shared = {
        "wq8": wchunk(Wq, _F8, WS),
        "wk8": wchunk(Wk, _F8, WS),
        "wvT": wchunk(Wv, _BF16),
        "woT": wchunk(Wo, _BF16),
        # DVE/ACT pre-add this to the WS-scaled q PSUM; ACT then multiplies
        # by SCALE/WS, so the bias carries WS (not SCALE).
        "bqs": np.ascontiguousarray((WS * bq).astype(np.float32).reshape(DC, P).T),
        "bqa": np.ascontiguousarray((SCALE * bq).astype(np.float32).reshape(DC, P).T),
        "bvc": np.ascontiguousarray((T * bv).astype(np.float32).reshape(DC, P).T),
        "bob": np.ascontiguousarray(bo.astype(np.float32).reshape(DC, P).T),
        "bvr": bv.astype(_BF16)[None, :],
        "one8": np.ones((P, 2, 16), _F8),
    }
    return shared


def _prep_x(xb):
    xT = np.ascontiguousarray(xb.T)  # [D, T]
    x8 = xb.astype(_F8)
    xr8 = (xb - x8.astype(np.float32)).astype(_F8)

    def tok(a):
        return np.ascontiguousarray(
            a.reshape(NG, 2, P, D).transpose(2, 0, 1, 3)
        )

    return {
        "xT8": np.ascontiguousarray(
            xT.reshape(DC, P, T).transpose(1, 0, 2)
        ).astype(_F8),
        "xt8": tok(x8),
        "xr8": tok(xr8),
    }


def _run(in_maps, trace=False, debug=False, **kw):
    from concourse.bass_utils import run_bass_kernel_spmd

    nc = _get_program(debug)
    return run_bass_kernel_spmd(nc, in_maps, list(range(len(in_maps))), trace=trace, **kw)


def kernel(x, Wq, bq, Wk, Wv, bv, Wo, bo):
    x = np.asarray(x, dtype=np.float32)
    assert x.shape == (B, T, D), x.shape
    shared = _prep_shared(
        np.asarray(Wq, np.float32), np.asarray(bq, np.float32),
        np.asarray(Wk, np.float32), np.asarray(Wv, np.float32),
        np.asarray(bv, np.float32), np.asarray(Wo, np.float32),
        np.asarray(bo, np.float32),
    )
    in_maps = []
    for b in range(B):
        m = dict(shared)
        m.update(_prep_x(x[b]))
        in_maps.append(m)

    res = _run(in_maps)
    out = np.empty((B, T, D), np.float32)
    for b in range(B):
        yT = np.asarray(res.results[b]["yT"]).astype(np.float32)  # [P, DC, T]
        out[b] = yT.transpose(1, 0, 2).reshape(D, T).T
    return out
